# revision 24
# baseline (speedup 1.0000x reference)
"""DiceLoss kernel for Trainium2 (8 NeuronCores, pure data parallel).

Problem: softmax over C=19 classes of predict [8, 19, 512, 512], one-hot of
target [8, 512, 512], then per-sample per-class sums
    psum[n,c]  = sum_pix softmax(x)[n,c,pix]
    inter[n,c] = sum_{pix: t=c} softmax(x)[n,c,pix]
    tsum[n,c]  = #{pix: t=c}
and dice = mean_c mean_n (1 - (2*inter+1)/(psum+tsum+1)).

Sharding: one sample per core (batch N=8 across 8 cores). Each core returns
[3*C] partial sums; the tiny final formula runs on host.

Device layout per core: x as [C, 128, 2048] bf16 (pixel-partition,
class-blocked free dim), processed in column chunks of F=512:
  - ScalarE: one Exp activation per chunk over all classes
  - DVE: pairwise-tree class sum -> denominator, reciprocal,
    then per class three fused ops:
      tensor_scalar(is_equal, accum)        -> one-hot mask + tsum column
      tensor_tensor_reduce(E*R, add-accum)  -> probs + psum column (chunk-chained)
      tensor_tensor_reduce(O*P, add-accum)  -> inter column (chunk-chained)
  - TensorE: final [128, 57] x ones matmul to reduce partitions.
Inputs are cast to bf16 on host (halves DMA bytes; fp32 accumulation of the
reductions keeps the loss error ~1e-4).
"""

import numpy as np
import ml_dtypes

N, C, H, W = 8, 19, 512, 512
PIX = H * W  # 262144
P = 128
FTOT = PIX // P  # 2048
F = 512
NCHUNK = FTOT // F
NCORES = 8

_PROG = None


def _build_program():
    from contextlib import ExitStack

    import concourse.bass as bass
    import concourse.tile as tile
    from concourse import mybir

    dt = mybir.dt
    Alu = mybir.AluOpType
    Act = mybir.ActivationFunctionType

    import bass_rust as _br

    class _TC(tile.TileContext):
        # This walrus build rejects instructions carrying more than ONE
        # sync-wait ("Too many sync wait commands", matching bass_rust's
        # inst_waits_full cap), but Tile's wait assignment occasionally
        # stacks 2-3 waits on one instruction and puts one wait per active
        # proc (12 here) on the tail drain. Hoist every excess wait onto an
        # injected same-engine single-wait Drain placed just before.

        def _split_excess_waits(self, insts):
            out = []
            for inst in insts:
                si = inst.sync_info
                if si is not None and len(si.on_wait) > 1:
                    moved = []
                    while len(si.on_wait) > 1:
                        moved.append(si.on_wait.pop())
                    for w in reversed(moved):
                        d = mybir.InstDrain(
                            name=self.nc.get_next_instruction_name(),
                            ins=[],
                            outs=[],
                            bass_is_fusable=False,
                        )
                        d.engine = inst.engine
                        d.sync_info = _br.SyncInfo(on_wait=[w], on_update=[])
                        self.nc.register_instruction(d)
                        out.append(d)
                out.append(inst)
            insts[:] = out

        def _lower_ordered_insts(self, ordered):
            for insts in ordered.values():
                self._split_excess_waits(insts)
            return super()._lower_ordered_insts(ordered)

        def _drain_and_barrier(self, tick_clock, wait_clock):
            from concourse.vector_clock import ScopedClock

            nc = self.nc
            drain_inst = nc.sync.drain()
            wait_clock.add_sem_waits(
                drain_inst.ins, ScopedClock({None: tick_clock.global_clock})
            )
            si = drain_inst.ins.sync_info
            moved = []
            while len(si.on_wait) > 1:
                moved.append(si.on_wait.pop())
            for w in moved:
                d2 = nc.sync.drain()
                d2.ins.sync_info = _br.SyncInfo(on_wait=[w], on_update=[])

            nc.all_engine_barrier()
            assert self.sems is not None
            popped = nc._tile_sem_poison_stack.pop()
            assert popped is self._sem_poison
            nc.clear_and_free_semaphores(list(self.sems.allocated().values()))
            nc.all_engine_barrier()

    nc = bass.Bass(
        "TRN2", target_bir_lowering=False, debug=False, num_devices=NCORES
    )
    x_d = nc.dram_tensor("x", [C, P, FTOT], dt.bfloat16, kind="ExternalInput").ap()
    t_d = nc.dram_tensor("t", [P, FTOT], dt.bfloat16, kind="ExternalInput").ap()
    out_d = nc.dram_tensor("out", [C, 2], dt.float32, kind="ExternalOutput").ap()

    with nc.allow_low_precision("bf16 softmax-stat kernel"), \
            _TC(nc) as tc, ExitStack() as ctx:
        # DMA-written tiles get one slot per chunk: a DMACopy may carry at
        # most ONE sync-wait on TRN2, so slot reuse (which adds WAR/WAW
        # waits onto the DMA) must be avoided for them.
        xp = ctx.enter_context(tc.tile_pool(name="xp", bufs=3))
        ep = ctx.enter_context(tc.tile_pool(name="ep", bufs=2))
        tp = ctx.enter_context(tc.tile_pool(name="tp", bufs=NCHUNK))
        dp = ctx.enter_context(tc.tile_pool(name="dp", bufs=2))
        sp = ctx.enter_context(tc.tile_pool(name="sp", bufs=3))
        cp = ctx.enter_context(tc.tile_pool(name="cp", bufs=1))
        pp = ctx.enter_context(tc.tile_pool(name="pp", bufs=1, space="PSUM"))

        # per-class one-hot lhsT columns: block c is a [P, C] matrix whose
        # column c is all-ones -> matmul with rhs [P, F] lands the
        # pixel-partition sums of rhs on PSUM partition c.
        cols = cp.tile([P, C * C], dt.bfloat16)
        nc.vector.memset(cols[:], 0.0)
        for c in range(C):
            nc.vector.memset(cols[:, c * C + c : c * C + c + 1], 1.0)

        ps_acc = pp.tile([C, F], dt.float32)
        in_acc = pp.tile([C, F], dt.float32)

        for j in range(NCHUNK):
            xt = xp.tile([P, C * F], dt.bfloat16, tag="x")
            xv = xt[:].rearrange("p (c f) -> p c f", c=C)
            et = ep.tile([P, C * F], dt.bfloat16, tag="e")
            ev = et[:].rearrange("p (c f) -> p c f", c=C)
            CSPLIT = 10
            for c0, c1 in ((0, CSPLIT), (CSPLIT, C)):
                nc.sync.dma_start(
                    out=xv[:, c0:c1, :],
                    in_=x_d[c0:c1, :, j * F : (j + 1) * F].rearrange(
                        "c p f -> p c f"
                    ),
                )
                nc.scalar.activation(
                    et[:, c0 * F : c1 * F], xt[:, c0 * F : c1 * F], Act.Exp
                )
            tt = tp.tile([P, F], dt.bfloat16, tag="t")
            nc.scalar.dma_start(out=tt[:], in_=t_d[:, j * F : (j + 1) * F])

            # denominator: tree-sum the 19 class blocks (bf16 2x mode),
            # last add in fp32 for reciprocal_approx_fast
            s9 = sp.tile([P, 9 * F], dt.bfloat16, tag="s9", bufs=1)
            s9v = s9[:].rearrange("p (c f) -> p c f", c=9)
            nc.vector.tensor_tensor(
                s9v[:, :, :], ev[:, 0:18:2, :], ev[:, 1:19:2, :], Alu.add
            )
            s4 = sp.tile([P, 4 * F], dt.bfloat16, tag="s4", bufs=1)
            s4v = s4[:].rearrange("p (c f) -> p c f", c=4)
            nc.vector.tensor_tensor(
                s4v[:, :, :], s9v[:, 0:8:2, :], s9v[:, 1:9:2, :], Alu.add
            )
            s2 = sp.tile([P, 2 * F], dt.bfloat16, tag="s2", bufs=1)
            s2v = s2[:].rearrange("p (c f) -> p c f", c=2)
            nc.vector.tensor_tensor(
                s2v[:, :, :], s4v[:, 0:4:2, :], s4v[:, 1:4:2, :], Alu.add
            )
            s1 = sp.tile([P, F], dt.bfloat16, tag="s1", bufs=1)
            nc.vector.tensor_tensor(s1[:], s2v[:, 0, :], s2v[:, 1, :], Alu.add)
            d0 = sp.tile([P, F], dt.bfloat16, tag="d0", bufs=1)
            nc.vector.tensor_tensor(d0[:], s1[:], s9v[:, 8, :], Alu.add)
            dd = sp.tile([P, F], dt.bfloat16, tag="dd", bufs=1)
            nc.vector.tensor_tensor(dd[:], d0[:], ev[:, 18, :], Alu.add)
            rt = dp.tile([P, F], dt.bfloat16, tag="r")
            nc.vector.reciprocal(rt[:], dd[:])

            for c in range(C):
                first = j == 0 and c == 0
                last = j == NCHUNK - 1 and c == C - 1
                lhs = cols[:, c * C : (c + 1) * C]
                o = sp.tile([P, F], dt.bfloat16, tag="o", bufs=12)
                nc.vector.tensor_scalar(o[:], tt[:], float(c), None, Alu.is_equal)
                pc = sp.tile([P, F], dt.bfloat16, tag="pc", bufs=12)
                nc.vector.tensor_tensor(pc[:], ev[:, c, :], rt[:], Alu.mult)
                oc = sp.tile([P, F], dt.bfloat16, tag="oc", bufs=12)
                nc.vector.tensor_tensor(oc[:], o[:], pc[:], Alu.mult)
                nc.tensor.matmul(
                    ps_acc[:], lhsT=lhs, rhs=pc[:], start=first, stop=last
                )
                nc.tensor.matmul(
                    in_acc[:], lhsT=lhs, rhs=oc[:], start=first, stop=last
                )

        # free-dim reduce of the three PSUM accumulators -> [C, 3] -> DRAM
        ob = cp.tile([C, 2], dt.float32)
        for k, acc in enumerate((ps_acc, in_acc)):
            nc.vector.tensor_reduce(
                out=ob[:, k : k + 1],
                in_=acc[:],
                axis=mybir.AxisListType.X,
                op=Alu.add,
            )
        nc.sync.dma_start(out=out_d[:], in_=ob[:])

    return nc


def _get_program():
    global _PROG
    if _PROG is None:
        _PROG = _build_program()
    return _PROG


def _shard_inputs(predict, target):
    x = np.ascontiguousarray(predict, dtype=np.float32).reshape(N, C, P, FTOT)
    x = x.astype(ml_dtypes.bfloat16)
    t = (
        np.ascontiguousarray(target)
        .reshape(N, P, FTOT)
        .astype(np.float32)
        .astype(ml_dtypes.bfloat16)
    )
    return [{"x": x[i], "t": t[i]} for i in range(N)]


def kernel(predict, target):
    from concourse.bass_utils import run_bass_kernel_spmd

    nc = _get_program()
    in_maps = _shard_inputs(predict, target)
    res = run_bass_kernel_spmd(nc, in_maps, list(range(NCORES)))
    stats = np.stack(
        [np.asarray(res.results[i]["out"], dtype=np.float32).reshape(C, 2) for i in range(NCORES)]
    )
    psum = stats[:, :, 0]
    inter = stats[:, :, 1]
    tgt = np.ascontiguousarray(target).reshape(N, PIX)
    tsum = np.stack(
        [np.bincount(tgt[i].astype(np.int64), minlength=C)[:C] for i in range(N)]
    ).astype(np.float32)
    top = 2.0 * inter + 1.0
    bot = psum + tsum + 1.0
    per_class = np.mean(1.0 - top / bot, axis=0, dtype=np.float32)
    return np.float32(per_class.sum() / C)


# revision 25
# speedup vs baseline: 1.0893x; 1.0893x over previous
"""DiceLoss kernel for Trainium2 (8 NeuronCores, pure data parallel).

Problem: softmax over C=19 classes of predict [8, 19, 512, 512], one-hot of
target [8, 512, 512], then per-sample per-class sums
    psum[n,c]  = sum_pix softmax(x)[n,c,pix]
    inter[n,c] = sum_{pix: t=c} softmax(x)[n,c,pix]
    tsum[n,c]  = #{pix: t=c}
and dice = mean_c mean_n (1 - (2*inter+1)/(psum+tsum+1)).

Sharding: one sample per core (batch N=8 across 8 cores). Each core returns
[3*C] partial sums; the tiny final formula runs on host.

Device layout per core: x as [C, 128, 2048] bf16 (pixel-partition,
class-blocked free dim), processed in column chunks of F=512:
  - ScalarE: one Exp activation per chunk over all classes
  - DVE: pairwise-tree class sum -> denominator, reciprocal,
    then per class three fused ops:
      tensor_scalar(is_equal, accum)        -> one-hot mask + tsum column
      tensor_tensor_reduce(E*R, add-accum)  -> probs + psum column (chunk-chained)
      tensor_tensor_reduce(O*P, add-accum)  -> inter column (chunk-chained)
  - TensorE: final [128, 57] x ones matmul to reduce partitions.
Inputs are cast to bf16 on host (halves DMA bytes; fp32 accumulation of the
reductions keeps the loss error ~1e-4).
"""

import numpy as np
import ml_dtypes

N, C, H, W = 8, 19, 512, 512
PIX = H * W  # 262144
P = 128
FTOT = PIX // P  # 2048
F = 512
NCHUNK = FTOT // F
NCORES = 8

_PROG = None


def _build_program():
    from contextlib import ExitStack

    import concourse.bass as bass
    import concourse.tile as tile
    from concourse import mybir

    dt = mybir.dt
    Alu = mybir.AluOpType
    Act = mybir.ActivationFunctionType

    import bass_rust as _br

    class _TC(tile.TileContext):
        # Stock Tile puts one sem-wait per active proc on the tail drain,
        # which this walrus rejects (>1 wait per instruction). Emit the
        # global-clock waits as single-wait drains instead; body
        # instructions are legalized by bass_rust.generate_event_semaphores
        # after the context exits.
        def _drain_and_barrier(self, tick_clock, wait_clock):
            from concourse.vector_clock import ScopedClock

            nc = self.nc
            drain_inst = nc.sync.drain()
            wait_clock.add_sem_waits(
                drain_inst.ins, ScopedClock({None: tick_clock.global_clock})
            )
            si = drain_inst.ins.sync_info
            moved = []
            while len(si.on_wait) > 1:
                moved.append(si.on_wait.pop())
            for w in moved:
                d2 = nc.sync.drain()
                d2.ins.sync_info = _br.SyncInfo(on_wait=[w], on_update=[])

            nc.all_engine_barrier()
            assert self.sems is not None
            popped = nc._tile_sem_poison_stack.pop()
            assert popped is self._sem_poison
            nc.clear_and_free_semaphores(list(self.sems.allocated().values()))
            nc.all_engine_barrier()

    nc = bass.Bass(
        "TRN2", target_bir_lowering=False, debug=False, num_devices=NCORES
    )
    x_d = nc.dram_tensor("x", [C, P, FTOT], dt.bfloat16, kind="ExternalInput").ap()
    t_d = nc.dram_tensor("t", [P, FTOT], dt.bfloat16, kind="ExternalInput").ap()
    out_d = nc.dram_tensor("out", [C, 2], dt.float32, kind="ExternalOutput").ap()

    with nc.allow_low_precision("bf16 softmax-stat kernel"), \
            _TC(nc) as tc, ExitStack() as ctx:
        # DMA-written tiles get one slot per chunk: a DMACopy may carry at
        # most ONE sync-wait on TRN2, so slot reuse (which adds WAR/WAW
        # waits onto the DMA) must be avoided for them.
        xp = ctx.enter_context(tc.tile_pool(name="xp", bufs=3))
        ep = ctx.enter_context(tc.tile_pool(name="ep", bufs=2))
        tp = ctx.enter_context(tc.tile_pool(name="tp", bufs=NCHUNK))
        dp = ctx.enter_context(tc.tile_pool(name="dp", bufs=2))
        sp = ctx.enter_context(tc.tile_pool(name="sp", bufs=3))
        cp = ctx.enter_context(tc.tile_pool(name="cp", bufs=1))
        pp = ctx.enter_context(tc.tile_pool(name="pp", bufs=1, space="PSUM"))

        # per-class one-hot lhsT columns: block c is a [P, C] matrix whose
        # column c is all-ones -> matmul with rhs [P, F] lands the
        # pixel-partition sums of rhs on PSUM partition c.
        cols = cp.tile([P, C * C], dt.bfloat16)
        nc.vector.memset(cols[:], 0.0)
        for c in range(C):
            nc.vector.memset(cols[:, c * C + c : c * C + c + 1], 1.0)

        ps_acc = pp.tile([C, F], dt.float32)
        in_acc = pp.tile([C, F], dt.float32)

        for j in range(NCHUNK):
            xt = xp.tile([P, C * F], dt.bfloat16, tag="x")
            xv = xt[:].rearrange("p (c f) -> p c f", c=C)
            et = ep.tile([P, C * F], dt.bfloat16, tag="e")
            ev = et[:].rearrange("p (c f) -> p c f", c=C)
            CSPLIT = 10
            for c0, c1 in ((0, CSPLIT), (CSPLIT, C)):
                nc.sync.dma_start(
                    out=xv[:, c0:c1, :],
                    in_=x_d[c0:c1, :, j * F : (j + 1) * F].rearrange(
                        "c p f -> p c f"
                    ),
                )
                nc.scalar.activation(
                    et[:, c0 * F : c1 * F], xt[:, c0 * F : c1 * F], Act.Exp
                )
            tt = tp.tile([P, F], dt.bfloat16, tag="t")
            nc.scalar.dma_start(out=tt[:], in_=t_d[:, j * F : (j + 1) * F])

            # denominator: tree-sum the 19 class blocks (bf16 2x mode),
            # last add in fp32 for reciprocal_approx_fast
            s9 = sp.tile([P, 9 * F], dt.bfloat16, tag="s9", bufs=1)
            s9v = s9[:].rearrange("p (c f) -> p c f", c=9)
            nc.vector.tensor_tensor(
                s9v[:, :, :], ev[:, 0:18:2, :], ev[:, 1:19:2, :], Alu.add
            )
            s4 = sp.tile([P, 4 * F], dt.bfloat16, tag="s4", bufs=1)
            s4v = s4[:].rearrange("p (c f) -> p c f", c=4)
            nc.vector.tensor_tensor(
                s4v[:, :, :], s9v[:, 0:8:2, :], s9v[:, 1:9:2, :], Alu.add
            )
            s2 = sp.tile([P, 2 * F], dt.bfloat16, tag="s2", bufs=1)
            s2v = s2[:].rearrange("p (c f) -> p c f", c=2)
            nc.vector.tensor_tensor(
                s2v[:, :, :], s4v[:, 0:4:2, :], s4v[:, 1:4:2, :], Alu.add
            )
            s1 = sp.tile([P, F], dt.bfloat16, tag="s1", bufs=1)
            nc.vector.tensor_tensor(s1[:], s2v[:, 0, :], s2v[:, 1, :], Alu.add)
            d0 = sp.tile([P, F], dt.bfloat16, tag="d0", bufs=1)
            nc.vector.tensor_tensor(d0[:], s1[:], s9v[:, 8, :], Alu.add)
            dd = sp.tile([P, F], dt.bfloat16, tag="dd", bufs=1)
            nc.vector.tensor_tensor(dd[:], d0[:], ev[:, 18, :], Alu.add)
            rt = dp.tile([P, F], dt.bfloat16, tag="r")
            nc.vector.reciprocal(rt[:], dd[:])

            for c in range(C):
                first = j == 0 and c == 0
                last = j == NCHUNK - 1 and c == C - 1
                lhs = cols[:, c * C : (c + 1) * C]
                o = sp.tile([P, F], dt.bfloat16, tag="o", bufs=12)
                nc.vector.tensor_scalar(o[:], tt[:], float(c), None, Alu.is_equal)
                pc = sp.tile([P, F], dt.bfloat16, tag="pc", bufs=12)
                nc.vector.tensor_tensor(pc[:], ev[:, c, :], rt[:], Alu.mult)
                oc = sp.tile([P, F], dt.bfloat16, tag="oc", bufs=12)
                nc.vector.tensor_tensor(oc[:], o[:], pc[:], Alu.mult)
                nc.tensor.matmul(
                    ps_acc[:], lhsT=lhs, rhs=pc[:], start=first, stop=last
                )
                nc.tensor.matmul(
                    in_acc[:], lhsT=lhs, rhs=oc[:], start=first, stop=last
                )

        # free-dim reduce of the three PSUM accumulators -> [C, 3] -> DRAM
        ob = cp.tile([C, 2], dt.float32)
        for k, acc in enumerate((ps_acc, in_acc)):
            nc.vector.tensor_reduce(
                out=ob[:, k : k + 1],
                in_=acc[:],
                axis=mybir.AxisListType.X,
                op=Alu.add,
            )
        nc.sync.dma_start(out=out_d[:], in_=ob[:])

    _br.move_matmul_waits_to_ldweights(nc.m)
    _br.generate_event_semaphores(nc)
    return nc


def _get_program():
    global _PROG
    if _PROG is None:
        _PROG = _build_program()
    return _PROG


def _shard_inputs(predict, target):
    x = np.ascontiguousarray(predict, dtype=np.float32).reshape(N, C, P, FTOT)
    x = x.astype(ml_dtypes.bfloat16)
    t = (
        np.ascontiguousarray(target)
        .reshape(N, P, FTOT)
        .astype(np.float32)
        .astype(ml_dtypes.bfloat16)
    )
    return [{"x": x[i], "t": t[i]} for i in range(N)]


def kernel(predict, target):
    from concourse.bass_utils import run_bass_kernel_spmd

    nc = _get_program()
    in_maps = _shard_inputs(predict, target)
    res = run_bass_kernel_spmd(nc, in_maps, list(range(NCORES)))
    stats = np.stack(
        [np.asarray(res.results[i]["out"], dtype=np.float32).reshape(C, 2) for i in range(NCORES)]
    )
    psum = stats[:, :, 0]
    inter = stats[:, :, 1]
    tgt = np.ascontiguousarray(target).reshape(N, PIX)
    tsum = np.stack(
        [np.bincount(tgt[i].astype(np.int64), minlength=C)[:C] for i in range(N)]
    ).astype(np.float32)
    top = 2.0 * inter + 1.0
    bot = psum + tsum + 1.0
    per_class = np.mean(1.0 - top / bot, axis=0, dtype=np.float32)
    return np.float32(per_class.sum() / C)


# revision 26
# speedup vs baseline: 1.1031x; 1.0127x over previous
"""DiceLoss kernel for Trainium2 (8 NeuronCores, pure data parallel).

Problem: softmax over C=19 classes of predict [8, 19, 512, 512], one-hot of
target [8, 512, 512], then per-sample per-class sums
    psum[n,c]  = sum_pix softmax(x)[n,c,pix]
    inter[n,c] = sum_{pix: t=c} softmax(x)[n,c,pix]
    tsum[n,c]  = #{pix: t=c}
and dice = mean_c mean_n (1 - (2*inter+1)/(psum+tsum+1)).

Sharding: one sample per core (batch N=8 across 8 cores). Each core returns
[3*C] partial sums; the tiny final formula runs on host.

Device layout per core: x as [C, 128, 2048] bf16 (pixel-partition,
class-blocked free dim), processed in column chunks of F=512:
  - ScalarE: one Exp activation per chunk over all classes
  - DVE: pairwise-tree class sum -> denominator, reciprocal,
    then per class three fused ops:
      tensor_scalar(is_equal, accum)        -> one-hot mask + tsum column
      tensor_tensor_reduce(E*R, add-accum)  -> probs + psum column (chunk-chained)
      tensor_tensor_reduce(O*P, add-accum)  -> inter column (chunk-chained)
  - TensorE: final [128, 57] x ones matmul to reduce partitions.
Inputs are cast to bf16 on host (halves DMA bytes; fp32 accumulation of the
reductions keeps the loss error ~1e-4).
"""

import numpy as np
import ml_dtypes

N, C, H, W = 8, 19, 512, 512
PIX = H * W  # 262144
P = 128
FTOT = PIX // P  # 2048
F = 512
NCHUNK = FTOT // F
NCORES = 8

_PROG = None


def _build_program():
    from contextlib import ExitStack

    import concourse.bass as bass
    import concourse.tile as tile
    from concourse import mybir

    dt = mybir.dt
    Alu = mybir.AluOpType
    Act = mybir.ActivationFunctionType

    import bass_rust as _br

    class _TC(tile.TileContext):
        # Stock Tile puts one sem-wait per active proc on the tail drain,
        # which this walrus rejects (>1 wait per instruction). Emit the
        # global-clock waits as single-wait drains instead; body
        # instructions are legalized by bass_rust.generate_event_semaphores
        # after the context exits.
        def _drain_and_barrier(self, tick_clock, wait_clock):
            from concourse.vector_clock import ScopedClock

            nc = self.nc
            drain_inst = nc.sync.drain()
            wait_clock.add_sem_waits(
                drain_inst.ins, ScopedClock({None: tick_clock.global_clock})
            )
            si = drain_inst.ins.sync_info
            moved = []
            while len(si.on_wait) > 1:
                moved.append(si.on_wait.pop())
            for w in moved:
                d2 = nc.sync.drain()
                d2.ins.sync_info = _br.SyncInfo(on_wait=[w], on_update=[])

            nc.all_engine_barrier()
            assert self.sems is not None
            popped = nc._tile_sem_poison_stack.pop()
            assert popped is self._sem_poison
            nc.clear_and_free_semaphores(list(self.sems.allocated().values()))
            nc.all_engine_barrier()

    nc = bass.Bass(
        "TRN2", target_bir_lowering=False, debug=False, num_devices=NCORES
    )
    x_d = nc.dram_tensor("x", [C, P, FTOT], dt.bfloat16, kind="ExternalInput").ap()
    t_d = nc.dram_tensor("t", [P, FTOT], dt.bfloat16, kind="ExternalInput").ap()
    out_d = nc.dram_tensor("out", [C, 2], dt.float32, kind="ExternalOutput").ap()

    with nc.allow_low_precision("bf16 softmax-stat kernel"), \
            _TC(nc) as tc, ExitStack() as ctx:
        # DMA-written tiles get one slot per chunk: a DMACopy may carry at
        # most ONE sync-wait on TRN2, so slot reuse (which adds WAR/WAW
        # waits onto the DMA) must be avoided for them.
        xp = ctx.enter_context(tc.tile_pool(name="xp", bufs=3))
        ep = ctx.enter_context(tc.tile_pool(name="ep", bufs=2))
        tp = ctx.enter_context(tc.tile_pool(name="tp", bufs=NCHUNK))
        dp = ctx.enter_context(tc.tile_pool(name="dp", bufs=2))
        sp = ctx.enter_context(tc.tile_pool(name="sp", bufs=3))
        cp = ctx.enter_context(tc.tile_pool(name="cp", bufs=1))
        pp = ctx.enter_context(tc.tile_pool(name="pp", bufs=1, space="PSUM"))

        # per-class one-hot lhsT columns: block c is a [P, C] matrix whose
        # column c is all-ones -> matmul with rhs [P, F] lands the
        # pixel-partition sums of rhs on PSUM partition c.
        cols = cp.tile([P, C * C], dt.bfloat16)
        nc.gpsimd.memset(cols[:], 0.0)
        for c in range(C):
            nc.gpsimd.memset(cols[:, c * C + c : c * C + c + 1], 1.0)

        ps_acc = pp.tile([C, F], dt.float32)
        in_acc = pp.tile([C, F], dt.float32)

        for j in range(NCHUNK):
            xt = xp.tile([P, C * F], dt.bfloat16, tag="x")
            xv = xt[:].rearrange("p (c f) -> p c f", c=C)
            et = ep.tile([P, C * F], dt.bfloat16, tag="e")
            ev = et[:].rearrange("p (c f) -> p c f", c=C)
            CSPLIT = 10
            for c0, c1 in ((0, CSPLIT), (CSPLIT, C)):
                nc.sync.dma_start(
                    out=xv[:, c0:c1, :],
                    in_=x_d[c0:c1, :, j * F : (j + 1) * F].rearrange(
                        "c p f -> p c f"
                    ),
                )
                nc.scalar.activation(
                    et[:, c0 * F : c1 * F], xt[:, c0 * F : c1 * F], Act.Exp
                )
            tt = tp.tile([P, F], dt.bfloat16, tag="t")
            nc.sync.dma_start(out=tt[:], in_=t_d[:, j * F : (j + 1) * F])

            # denominator: tree-sum the 19 class blocks (bf16 2x mode),
            # last add in fp32 for reciprocal_approx_fast
            s9 = sp.tile([P, 9 * F], dt.bfloat16, tag="s9", bufs=1)
            s9v = s9[:].rearrange("p (c f) -> p c f", c=9)
            nc.vector.tensor_tensor(
                s9v[:, :, :], ev[:, 0:18:2, :], ev[:, 1:19:2, :], Alu.add
            )
            s4 = sp.tile([P, 4 * F], dt.bfloat16, tag="s4", bufs=1)
            s4v = s4[:].rearrange("p (c f) -> p c f", c=4)
            nc.vector.tensor_tensor(
                s4v[:, :, :], s9v[:, 0:8:2, :], s9v[:, 1:9:2, :], Alu.add
            )
            s2 = sp.tile([P, 2 * F], dt.bfloat16, tag="s2", bufs=1)
            s2v = s2[:].rearrange("p (c f) -> p c f", c=2)
            nc.vector.tensor_tensor(
                s2v[:, :, :], s4v[:, 0:4:2, :], s4v[:, 1:4:2, :], Alu.add
            )
            s1 = sp.tile([P, F], dt.bfloat16, tag="s1", bufs=1)
            nc.vector.tensor_tensor(s1[:], s2v[:, 0, :], s2v[:, 1, :], Alu.add)
            d0 = sp.tile([P, F], dt.bfloat16, tag="d0", bufs=1)
            nc.vector.tensor_tensor(d0[:], s1[:], s9v[:, 8, :], Alu.add)
            dd = sp.tile([P, F], dt.bfloat16, tag="dd", bufs=1)
            nc.vector.tensor_tensor(dd[:], d0[:], ev[:, 18, :], Alu.add)
            rt = dp.tile([P, F], dt.bfloat16, tag="r")
            nc.vector.reciprocal(rt[:], dd[:])

            for c in range(C):
                first = j == 0 and c == 0
                last = j == NCHUNK - 1 and c == C - 1
                lhs = cols[:, c * C : (c + 1) * C]
                o = sp.tile([P, F], dt.bfloat16, tag="o", bufs=12)
                nc.vector.tensor_scalar(o[:], tt[:], float(c), None, Alu.is_equal)
                pc = sp.tile([P, F], dt.bfloat16, tag="pc", bufs=12)
                nc.vector.tensor_tensor(pc[:], ev[:, c, :], rt[:], Alu.mult)
                oc = sp.tile([P, F], dt.bfloat16, tag="oc", bufs=12)
                nc.vector.tensor_tensor(oc[:], o[:], pc[:], Alu.mult)
                nc.tensor.matmul(
                    ps_acc[:], lhsT=lhs, rhs=pc[:], start=first, stop=last
                )
                nc.tensor.matmul(
                    in_acc[:], lhsT=lhs, rhs=oc[:], start=first, stop=last
                )

        # free-dim reduce of the three PSUM accumulators -> [C, 3] -> DRAM
        ob = cp.tile([C, 2], dt.float32)
        for k, acc in enumerate((ps_acc, in_acc)):
            nc.vector.tensor_reduce(
                out=ob[:, k : k + 1],
                in_=acc[:],
                axis=mybir.AxisListType.X,
                op=Alu.add,
            )
        nc.sync.dma_start(out=out_d[:], in_=ob[:])

    _br.move_matmul_waits_to_ldweights(nc.m)
    _br.generate_event_semaphores(nc)
    return nc


def _get_program():
    global _PROG
    if _PROG is None:
        _PROG = _build_program()
    return _PROG


def _shard_inputs(predict, target):
    x = np.ascontiguousarray(predict, dtype=np.float32).reshape(N, C, P, FTOT)
    x = x.astype(ml_dtypes.bfloat16)
    t = (
        np.ascontiguousarray(target)
        .reshape(N, P, FTOT)
        .astype(np.float32)
        .astype(ml_dtypes.bfloat16)
    )
    return [{"x": x[i], "t": t[i]} for i in range(N)]


def kernel(predict, target):
    from concourse.bass_utils import run_bass_kernel_spmd

    nc = _get_program()
    in_maps = _shard_inputs(predict, target)
    res = run_bass_kernel_spmd(nc, in_maps, list(range(NCORES)))
    stats = np.stack(
        [np.asarray(res.results[i]["out"], dtype=np.float32).reshape(C, 2) for i in range(NCORES)]
    )
    psum = stats[:, :, 0]
    inter = stats[:, :, 1]
    tgt = np.ascontiguousarray(target).reshape(N, PIX)
    tsum = np.stack(
        [np.bincount(tgt[i].astype(np.int64), minlength=C)[:C] for i in range(N)]
    ).astype(np.float32)
    top = 2.0 * inter + 1.0
    bot = psum + tsum + 1.0
    per_class = np.mean(1.0 - top / bot, axis=0, dtype=np.float32)
    return np.float32(per_class.sum() / C)


# revision 27
# speedup vs baseline: 1.1278x; 1.0224x over previous
"""DiceLoss kernel for Trainium2 (8 NeuronCores, pure data parallel).

Problem: softmax over C=19 classes of predict [8, 19, 512, 512], one-hot of
target [8, 512, 512], then per-sample per-class sums
    psum[n,c]  = sum_pix softmax(x)[n,c,pix]
    inter[n,c] = sum_{pix: t=c} softmax(x)[n,c,pix]
    tsum[n,c]  = #{pix: t=c}
and dice = mean_c mean_n (1 - (2*inter+1)/(psum+tsum+1)).

Sharding: one sample per core (batch N=8 across 8 cores). Each core returns
[3*C] partial sums; the tiny final formula runs on host.

Device layout per core: x as [C, 128, 2048] bf16 (pixel-partition,
class-blocked free dim), processed in column chunks of F=512:
  - ScalarE: one Exp activation per chunk over all classes
  - DVE: pairwise-tree class sum -> denominator, reciprocal,
    then per class three fused ops:
      tensor_scalar(is_equal, accum)        -> one-hot mask + tsum column
      tensor_tensor_reduce(E*R, add-accum)  -> probs + psum column (chunk-chained)
      tensor_tensor_reduce(O*P, add-accum)  -> inter column (chunk-chained)
  - TensorE: final [128, 57] x ones matmul to reduce partitions.
Inputs are cast to bf16 on host (halves DMA bytes; fp32 accumulation of the
reductions keeps the loss error ~1e-4).
"""

import numpy as np
import ml_dtypes

N, C, H, W = 8, 19, 512, 512
PIX = H * W  # 262144
P = 128
FTOT = PIX // P  # 2048
F = 512
NCHUNK = FTOT // F
NCORES = 8

_PROG = None


def _build_program():
    from contextlib import ExitStack

    import concourse.bass as bass
    import concourse.tile as tile
    from concourse import mybir

    dt = mybir.dt
    Alu = mybir.AluOpType
    Act = mybir.ActivationFunctionType

    import bass_rust as _br

    class _TC(tile.TileContext):
        # Stock Tile puts one sem-wait per active proc on the tail drain,
        # which this walrus rejects (>1 wait per instruction). Emit the
        # global-clock waits as single-wait drains instead; body
        # instructions are legalized by bass_rust.generate_event_semaphores
        # after the context exits.
        def _drain_and_barrier(self, tick_clock, wait_clock):
            from concourse.vector_clock import ScopedClock

            nc = self.nc
            drain_inst = nc.sync.drain()
            wait_clock.add_sem_waits(
                drain_inst.ins, ScopedClock({None: tick_clock.global_clock})
            )
            si = drain_inst.ins.sync_info
            moved = []
            while len(si.on_wait) > 1:
                moved.append(si.on_wait.pop())
            for w in moved:
                d2 = nc.sync.drain()
                d2.ins.sync_info = _br.SyncInfo(on_wait=[w], on_update=[])

            nc.all_engine_barrier()
            assert self.sems is not None
            popped = nc._tile_sem_poison_stack.pop()
            assert popped is self._sem_poison
            nc.clear_and_free_semaphores(list(self.sems.allocated().values()))
            nc.all_engine_barrier()

    nc = bass.Bass(
        "TRN2", target_bir_lowering=False, debug=False, num_devices=NCORES
    )
    x_d = nc.dram_tensor("x", [C, P, FTOT], dt.bfloat16, kind="ExternalInput").ap()
    t_d = nc.dram_tensor("t", [P, FTOT], dt.bfloat16, kind="ExternalInput").ap()
    out_d = nc.dram_tensor("out", [C, 2], dt.float32, kind="ExternalOutput").ap()

    with nc.allow_low_precision("bf16 softmax-stat kernel"), \
            _TC(nc) as tc, ExitStack() as ctx:
        # DMA-written tiles get one slot per chunk: a DMACopy may carry at
        # most ONE sync-wait on TRN2, so slot reuse (which adds WAR/WAW
        # waits onto the DMA) must be avoided for them.
        xp = ctx.enter_context(tc.tile_pool(name="xp", bufs=3))
        ep = ctx.enter_context(tc.tile_pool(name="ep", bufs=2))
        tp = ctx.enter_context(tc.tile_pool(name="tp", bufs=NCHUNK))
        dp = ctx.enter_context(tc.tile_pool(name="dp", bufs=2))
        sp = ctx.enter_context(tc.tile_pool(name="sp", bufs=3))
        cp = ctx.enter_context(tc.tile_pool(name="cp", bufs=1))
        pp = ctx.enter_context(tc.tile_pool(name="pp", bufs=1, space="PSUM"))

        # per-class one-hot lhsT columns: block c is a [P, C] matrix whose
        # column c is all-ones -> matmul with rhs [P, F] lands the
        # pixel-partition sums of rhs on PSUM partition c.
        cols = cp.tile([P, C * C], dt.bfloat16)
        nc.gpsimd.memset(cols[:], 0.0)
        for c in range(C):
            nc.gpsimd.memset(cols[:, c * C + c : c * C + c + 1], 1.0)

        ps_acc = pp.tile([C, F], dt.float32)
        in_acc = pp.tile([C, F], dt.float32)

        for j in range(NCHUNK):
            xt = xp.tile([P, C * F], dt.bfloat16, tag="x")
            xv = xt[:].rearrange("p (c f) -> p c f", c=C)
            et = ep.tile([P, C * F], dt.bfloat16, tag="e")
            ev = et[:].rearrange("p (c f) -> p c f", c=C)
            CSPLIT = 10
            for c0, c1 in ((0, CSPLIT), (CSPLIT, C)):
                nc.sync.dma_start(
                    out=xv[:, c0:c1, :],
                    in_=x_d[c0:c1, :, j * F : (j + 1) * F].rearrange(
                        "c p f -> p c f"
                    ),
                )
                nc.scalar.activation(
                    et[:, c0 * F : c1 * F], xt[:, c0 * F : c1 * F], Act.Exp
                )
            tt = tp.tile([P, F], dt.bfloat16, tag="t")
            nc.sync.dma_start(out=tt[:], in_=t_d[:, j * F : (j + 1) * F])

            # denominator: tree-sum the 19 class blocks (bf16 2x mode),
            # last add in fp32 for reciprocal_approx_fast
            s9 = sp.tile([P, 9 * F], dt.bfloat16, tag="s9", bufs=1)
            s9v = s9[:].rearrange("p (c f) -> p c f", c=9)
            nc.vector.tensor_tensor(
                s9v[:, :, :], ev[:, 0:18:2, :], ev[:, 1:19:2, :], Alu.add
            )
            s4 = sp.tile([P, 4 * F], dt.bfloat16, tag="s4", bufs=1)
            s4v = s4[:].rearrange("p (c f) -> p c f", c=4)
            nc.vector.tensor_tensor(
                s4v[:, :, :], s9v[:, 0:8:2, :], s9v[:, 1:9:2, :], Alu.add
            )
            s2 = sp.tile([P, 2 * F], dt.bfloat16, tag="s2", bufs=1)
            s2v = s2[:].rearrange("p (c f) -> p c f", c=2)
            nc.vector.tensor_tensor(
                s2v[:, :, :], s4v[:, 0:4:2, :], s4v[:, 1:4:2, :], Alu.add
            )
            s1 = sp.tile([P, F], dt.bfloat16, tag="s1", bufs=1)
            nc.vector.tensor_tensor(s1[:], s2v[:, 0, :], s2v[:, 1, :], Alu.add)
            d0 = sp.tile([P, F], dt.bfloat16, tag="d0", bufs=1)
            nc.vector.tensor_tensor(d0[:], s1[:], s9v[:, 8, :], Alu.add)
            dd = sp.tile([P, F], dt.bfloat16, tag="dd", bufs=1)
            nc.vector.tensor_tensor(dd[:], d0[:], ev[:, 18, :], Alu.add)
            rt = dp.tile([P, F], dt.bfloat16, tag="r")
            nc.vector.reciprocal(rt[:], dd[:])

            ot = sp.tile([P, C * F], dt.bfloat16, tag="ot", bufs=2)
            for c in range(C):
                nc.vector.tensor_scalar(
                    ot[:, c * F : (c + 1) * F], tt[:], float(c), None, Alu.is_equal
                )
            for c in range(C):
                first = j == 0 and c == 0
                last = j == NCHUNK - 1 and c == C - 1
                lhs = cols[:, c * C : (c + 1) * C]
                o = ot[:, c * F : (c + 1) * F]
                pc = sp.tile([P, F], dt.bfloat16, tag="pc", bufs=12)
                nc.vector.tensor_tensor(pc[:], ev[:, c, :], rt[:], Alu.mult)
                oc = sp.tile([P, F], dt.bfloat16, tag="oc", bufs=12)
                nc.vector.tensor_tensor(oc[:], o[:], pc[:], Alu.mult)
                nc.tensor.matmul(
                    ps_acc[:], lhsT=lhs, rhs=pc[:], start=first, stop=last
                )
                nc.tensor.matmul(
                    in_acc[:], lhsT=lhs, rhs=oc[:], start=first, stop=last
                )

        # free-dim reduce of the three PSUM accumulators -> [C, 3] -> DRAM
        ob = cp.tile([C, 2], dt.float32)
        for k, acc in enumerate((ps_acc, in_acc)):
            nc.vector.tensor_reduce(
                out=ob[:, k : k + 1],
                in_=acc[:],
                axis=mybir.AxisListType.X,
                op=Alu.add,
            )
        nc.sync.dma_start(out=out_d[:], in_=ob[:])

    _br.move_matmul_waits_to_ldweights(nc.m)
    _br.generate_event_semaphores(nc)
    return nc


def _get_program():
    global _PROG
    if _PROG is None:
        _PROG = _build_program()
    return _PROG


def _shard_inputs(predict, target):
    x = np.ascontiguousarray(predict, dtype=np.float32).reshape(N, C, P, FTOT)
    x = x.astype(ml_dtypes.bfloat16)
    t = (
        np.ascontiguousarray(target)
        .reshape(N, P, FTOT)
        .astype(np.float32)
        .astype(ml_dtypes.bfloat16)
    )
    return [{"x": x[i], "t": t[i]} for i in range(N)]


def kernel(predict, target):
    from concourse.bass_utils import run_bass_kernel_spmd

    nc = _get_program()
    in_maps = _shard_inputs(predict, target)
    res = run_bass_kernel_spmd(nc, in_maps, list(range(NCORES)))
    stats = np.stack(
        [np.asarray(res.results[i]["out"], dtype=np.float32).reshape(C, 2) for i in range(NCORES)]
    )
    psum = stats[:, :, 0]
    inter = stats[:, :, 1]
    tgt = np.ascontiguousarray(target).reshape(N, PIX)
    tsum = np.stack(
        [np.bincount(tgt[i].astype(np.int64), minlength=C)[:C] for i in range(N)]
    ).astype(np.float32)
    top = 2.0 * inter + 1.0
    bot = psum + tsum + 1.0
    per_class = np.mean(1.0 - top / bot, axis=0, dtype=np.float32)
    return np.float32(per_class.sum() / C)


# revision 29
# speedup vs baseline: 1.1404x; 1.0111x over previous
"""DiceLoss kernel for Trainium2 (8 NeuronCores, pure data parallel).

Problem: softmax over C=19 classes of predict [8, 19, 512, 512], one-hot of
target [8, 512, 512], then per-sample per-class sums
    psum[n,c]  = sum_pix softmax(x)[n,c,pix]
    inter[n,c] = sum_{pix: t=c} softmax(x)[n,c,pix]
    tsum[n,c]  = #{pix: t=c}
and dice = mean_c mean_n (1 - (2*inter+1)/(psum+tsum+1)).

Sharding: one sample per core (batch N=8 across 8 cores). Each core returns
[3*C] partial sums; the tiny final formula runs on host.

Device layout per core: x as [C, 128, 2048] bf16 (pixel-partition,
class-blocked free dim), processed in column chunks of F=512:
  - ScalarE: Exp activation (two class-group halves per chunk, pipelined
    behind the split DMA)
  - DVE: per-chunk one-hot masks (tensor_scalar is_equal, 4x mode, dep only
    on the tiny t tile so they fill the DMA/exp head), pairwise-tree class
    sum -> denominator (bf16 2x mode), reciprocal, then per class two bf16
    2x-mode products: P = E*R and OP = O*P
  - TensorE: per class a [128,19] one-hot-column lhsT matmul against
    rhs P / OP accumulates the pixel-partition sums for psum / inter into
    two [19, F] PSUM banks (start only on the very first matmul); a final
    free-dim reduce emits [C, 2] per core.
tsum is the exact integer histogram of the target input, computed on host
during sharding. Inputs are cast to bf16 on host (halves DMA bytes; fp32
PSUM accumulation keeps the loss error ~1e-6).

Hardware quirks worked around here: this walrus build allows at most ONE
sync-wait per instruction (two on InstEventSemaphore) -> tail-drain waits
are emitted as single-wait drains and the body is legalized with
bass_rust.generate_event_semaphores; InstISA-encoded DVE ops
(tensor_tensor_reduce, reciprocal_approx_*) fail codegen ("ISA wrong
length") and are avoided; gpsimd tensor ops measure ~10x slower than the
cost model and SWDGE DMAs add a ~30us Pool dge-drain to the tail, so all
DMAs go through SP HWDGE and gpsimd only does constant memsets.

Measured on trn2 via axon: HW exec ~124.3us per core (8 cores SPMD),
relative error vs fp32 reference ~6e-7. DVE-bound at ~86% occupancy.
"""

import numpy as np
import ml_dtypes

N, C, H, W = 8, 19, 512, 512
PIX = H * W  # 262144
P = 128
FTOT = PIX // P  # 2048
F = 512
NCHUNK = FTOT // F
NCORES = 8

_PROG = None


def _build_program():
    from contextlib import ExitStack

    import concourse.bass as bass
    import concourse.tile as tile
    from concourse import mybir

    dt = mybir.dt
    Alu = mybir.AluOpType
    Act = mybir.ActivationFunctionType

    import bass_rust as _br

    class _TC(tile.TileContext):
        # Stock Tile puts one sem-wait per active proc on the tail drain,
        # which this walrus rejects (>1 wait per instruction). Emit the
        # global-clock waits as single-wait drains instead; body
        # instructions are legalized by bass_rust.generate_event_semaphores
        # after the context exits.
        def _drain_and_barrier(self, tick_clock, wait_clock):
            from concourse.vector_clock import ScopedClock

            nc = self.nc
            drain_inst = nc.sync.drain()
            wait_clock.add_sem_waits(
                drain_inst.ins, ScopedClock({None: tick_clock.global_clock})
            )
            si = drain_inst.ins.sync_info
            moved = []
            while len(si.on_wait) > 1:
                moved.append(si.on_wait.pop())
            for w in moved:
                d2 = nc.sync.drain()
                d2.ins.sync_info = _br.SyncInfo(on_wait=[w], on_update=[])

            nc.all_engine_barrier()
            assert self.sems is not None
            popped = nc._tile_sem_poison_stack.pop()
            assert popped is self._sem_poison
            nc.clear_and_free_semaphores(list(self.sems.allocated().values()))
            nc.all_engine_barrier()

    nc = bass.Bass(
        "TRN2", target_bir_lowering=False, debug=False, num_devices=NCORES
    )
    x_d = nc.dram_tensor("x", [C, P, FTOT], dt.bfloat16, kind="ExternalInput").ap()
    t_d = nc.dram_tensor("t", [P, FTOT], dt.bfloat16, kind="ExternalInput").ap()
    out_d = nc.dram_tensor("out", [C, 2], dt.float32, kind="ExternalOutput").ap()

    with nc.allow_low_precision("bf16 softmax-stat kernel"), \
            _TC(nc) as tc, ExitStack() as ctx:
        # DMA-written tiles get one slot per chunk: a DMACopy may carry at
        # most ONE sync-wait on TRN2, so slot reuse (which adds WAR/WAW
        # waits onto the DMA) must be avoided for them.
        xp = ctx.enter_context(tc.tile_pool(name="xp", bufs=3))
        ep = ctx.enter_context(tc.tile_pool(name="ep", bufs=2))
        tp = ctx.enter_context(tc.tile_pool(name="tp", bufs=NCHUNK))
        dp = ctx.enter_context(tc.tile_pool(name="dp", bufs=2))
        sp = ctx.enter_context(tc.tile_pool(name="sp", bufs=3))
        cp = ctx.enter_context(tc.tile_pool(name="cp", bufs=1))
        pp = ctx.enter_context(tc.tile_pool(name="pp", bufs=1, space="PSUM"))

        # per-class one-hot lhsT columns: block c is a [P, C] matrix whose
        # column c is all-ones -> matmul with rhs [P, F] lands the
        # pixel-partition sums of rhs on PSUM partition c.
        cols = cp.tile([P, C * C], dt.bfloat16)
        nc.gpsimd.memset(cols[:], 0.0)
        for c in range(C):
            nc.gpsimd.memset(cols[:, c * C + c : c * C + c + 1], 1.0)

        ps_acc = pp.tile([C, F], dt.float32)
        in_acc = pp.tile([C, F], dt.float32)

        for j in range(NCHUNK):
            # t first: it is tiny and the 19 mask ops depend only on it,
            # so DVE can chew masks while the big x transfer streams in.
            tt = tp.tile([P, F], dt.bfloat16, tag="t")
            nc.sync.dma_start(out=tt[:], in_=t_d[:, j * F : (j + 1) * F])
            ot = sp.tile([P, C * F], dt.bfloat16, tag="ot", bufs=2)
            for c in range(C):
                nc.vector.tensor_scalar(
                    ot[:, c * F : (c + 1) * F], tt[:], float(c), None, Alu.is_equal
                )
            xt = xp.tile([P, C * F], dt.bfloat16, tag="x")
            xv = xt[:].rearrange("p (c f) -> p c f", c=C)
            et = ep.tile([P, C * F], dt.bfloat16, tag="e")
            ev = et[:].rearrange("p (c f) -> p c f", c=C)
            CSPLIT = 10
            for c0, c1 in ((0, CSPLIT), (CSPLIT, C)):
                nc.sync.dma_start(
                    out=xv[:, c0:c1, :],
                    in_=x_d[c0:c1, :, j * F : (j + 1) * F].rearrange(
                        "c p f -> p c f"
                    ),
                )
                nc.scalar.activation(
                    et[:, c0 * F : c1 * F], xt[:, c0 * F : c1 * F], Act.Exp
                )

            # denominator: tree-sum the 19 class blocks (bf16 2x mode),
            # last add in fp32 for reciprocal_approx_fast
            s9 = sp.tile([P, 9 * F], dt.bfloat16, tag="s9", bufs=1)
            s9v = s9[:].rearrange("p (c f) -> p c f", c=9)
            nc.vector.tensor_tensor(
                s9v[:, :, :], ev[:, 0:18:2, :], ev[:, 1:19:2, :], Alu.add
            )
            s4 = sp.tile([P, 4 * F], dt.bfloat16, tag="s4", bufs=1)
            s4v = s4[:].rearrange("p (c f) -> p c f", c=4)
            nc.vector.tensor_tensor(
                s4v[:, :, :], s9v[:, 0:8:2, :], s9v[:, 1:9:2, :], Alu.add
            )
            s2 = sp.tile([P, 2 * F], dt.bfloat16, tag="s2", bufs=1)
            s2v = s2[:].rearrange("p (c f) -> p c f", c=2)
            nc.vector.tensor_tensor(
                s2v[:, :, :], s4v[:, 0:4:2, :], s4v[:, 1:4:2, :], Alu.add
            )
            s1 = sp.tile([P, F], dt.bfloat16, tag="s1", bufs=1)
            nc.vector.tensor_tensor(s1[:], s2v[:, 0, :], s2v[:, 1, :], Alu.add)
            d0 = sp.tile([P, F], dt.bfloat16, tag="d0", bufs=1)
            nc.vector.tensor_tensor(d0[:], s1[:], s9v[:, 8, :], Alu.add)
            dd = sp.tile([P, F], dt.bfloat16, tag="dd", bufs=1)
            nc.vector.tensor_tensor(dd[:], d0[:], ev[:, 18, :], Alu.add)
            rt = dp.tile([P, F], dt.bfloat16, tag="r")
            nc.vector.reciprocal(rt[:], dd[:])

            for c in range(C):
                first = j == 0 and c == 0
                last = j == NCHUNK - 1 and c == C - 1
                lhs = cols[:, c * C : (c + 1) * C]
                o = ot[:, c * F : (c + 1) * F]
                pc = sp.tile([P, F], dt.bfloat16, tag="pc", bufs=12)
                nc.vector.tensor_tensor(pc[:], ev[:, c, :], rt[:], Alu.mult)
                oc = sp.tile([P, F], dt.bfloat16, tag="oc", bufs=12)
                nc.vector.tensor_tensor(oc[:], o[:], pc[:], Alu.mult)
                nc.tensor.matmul(
                    ps_acc[:], lhsT=lhs, rhs=pc[:], start=first, stop=last
                )
                nc.tensor.matmul(
                    in_acc[:], lhsT=lhs, rhs=oc[:], start=first, stop=last
                )

        # free-dim reduce of the three PSUM accumulators -> [C, 3] -> DRAM
        ob = cp.tile([C, 2], dt.float32)
        for k, acc in enumerate((ps_acc, in_acc)):
            nc.vector.tensor_reduce(
                out=ob[:, k : k + 1],
                in_=acc[:],
                axis=mybir.AxisListType.X,
                op=Alu.add,
            )
        nc.sync.dma_start(out=out_d[:], in_=ob[:])

    _br.move_matmul_waits_to_ldweights(nc.m)
    _br.generate_event_semaphores(nc)
    return nc


def _get_program():
    global _PROG
    if _PROG is None:
        _PROG = _build_program()
    return _PROG


def _shard_inputs(predict, target):
    x = np.ascontiguousarray(predict, dtype=np.float32).reshape(N, C, P, FTOT)
    x = x.astype(ml_dtypes.bfloat16)
    t = (
        np.ascontiguousarray(target)
        .reshape(N, P, FTOT)
        .astype(np.float32)
        .astype(ml_dtypes.bfloat16)
    )
    return [{"x": x[i], "t": t[i]} for i in range(N)]


def kernel(predict, target):
    from concourse.bass_utils import run_bass_kernel_spmd

    nc = _get_program()
    in_maps = _shard_inputs(predict, target)
    res = run_bass_kernel_spmd(nc, in_maps, list(range(NCORES)))
    stats = np.stack(
        [np.asarray(res.results[i]["out"], dtype=np.float32).reshape(C, 2) for i in range(NCORES)]
    )
    psum = stats[:, :, 0]
    inter = stats[:, :, 1]
    tgt = np.ascontiguousarray(target).reshape(N, PIX)
    tsum = np.stack(
        [np.bincount(tgt[i].astype(np.int64), minlength=C)[:C] for i in range(N)]
    ).astype(np.float32)
    top = 2.0 * inter + 1.0
    bot = psum + tsum + 1.0
    per_class = np.mean(1.0 - top / bot, axis=0, dtype=np.float32)
    return np.float32(per_class.sum() / C)


# revision 32
# speedup vs baseline: 1.1517x; 1.0099x over previous
"""DiceLoss kernel for Trainium2 (8 NeuronCores, pure data parallel).

Problem: softmax over C=19 classes of predict [8, 19, 512, 512], one-hot of
target [8, 512, 512], then per-sample per-class sums
    psum[n,c]  = sum_pix softmax(x)[n,c,pix]
    inter[n,c] = sum_{pix: t=c} softmax(x)[n,c,pix]
    tsum[n,c]  = #{pix: t=c}
and dice = mean_c mean_n (1 - (2*inter+1)/(psum+tsum+1)).

Sharding: one sample per core (batch N=8 across 8 cores). Each core returns
[3*C] partial sums; the tiny final formula runs on host.

Device layout per core: x as [C, 128, 2048] bf16 (pixel-partition,
class-blocked free dim), processed in column chunks of F=512:
  - ScalarE: Exp activation (two class-group halves per chunk, pipelined
    behind the split DMA)
  - DVE: per-chunk one-hot masks (tensor_scalar is_equal, 4x mode, dep only
    on the tiny t tile so they fill the DMA/exp head), pairwise-tree class
    sum -> denominator (bf16 2x mode), reciprocal, then per class two bf16
    2x-mode products: P = E*R and OP = O*P
  - TensorE: per class a [128,19] one-hot-column lhsT matmul against
    rhs P / OP accumulates the pixel-partition sums for psum / inter into
    two [19, F] PSUM banks (start only on the very first matmul); a final
    free-dim reduce emits [C, 2] per core.
tsum is the exact integer histogram of the target input, computed on host
during sharding. Inputs are cast to bf16 on host (halves DMA bytes; fp32
PSUM accumulation keeps the loss error ~1e-6).

Hardware quirks worked around here: this walrus build allows at most ONE
sync-wait per instruction (two on InstEventSemaphore) -> tail-drain waits
are emitted as single-wait drains and the body is legalized with
bass_rust.generate_event_semaphores; InstISA-encoded DVE ops
(tensor_tensor_reduce, reciprocal_approx_*) fail codegen ("ISA wrong
length") and are avoided; gpsimd tensor ops measure ~10x slower than the
cost model and SWDGE DMAs add a ~30us Pool dge-drain to the tail, so all
DMAs go through SP HWDGE and gpsimd only does constant memsets.

Measured on trn2 via axon: HW exec ~123us per core (8 cores SPMD),
relative error vs fp32 reference ~6e-7. DVE-bound at ~88% occupancy.
"""

import numpy as np
import ml_dtypes

N, C, H, W = 8, 19, 512, 512
PIX = H * W  # 262144
P = 128
FTOT = PIX // P  # 2048
F = 512
NCHUNK = FTOT // F
NCORES = 8

_PROG = None


def _build_program():
    from contextlib import ExitStack

    import concourse.bass as bass
    import concourse.tile as tile
    from concourse import mybir

    dt = mybir.dt
    Alu = mybir.AluOpType
    Act = mybir.ActivationFunctionType

    import bass_rust as _br

    class _TC(tile.TileContext):
        # Stock Tile puts one sem-wait per active proc on the tail drain,
        # which this walrus rejects (>1 wait per instruction). Emit the
        # global-clock waits as single-wait drains instead; body
        # instructions are legalized by bass_rust.generate_event_semaphores
        # after the context exits.
        def _drain_and_barrier(self, tick_clock, wait_clock):
            from concourse.vector_clock import ScopedClock

            nc = self.nc
            drain_inst = nc.sync.drain()
            wait_clock.add_sem_waits(
                drain_inst.ins, ScopedClock({None: tick_clock.global_clock})
            )
            si = drain_inst.ins.sync_info
            moved = []
            while len(si.on_wait) > 1:
                moved.append(si.on_wait.pop())
            for w in moved:
                d2 = nc.sync.drain()
                d2.ins.sync_info = _br.SyncInfo(on_wait=[w], on_update=[])

            nc.all_engine_barrier()
            assert self.sems is not None
            popped = nc._tile_sem_poison_stack.pop()
            assert popped is self._sem_poison
            nc.clear_and_free_semaphores(list(self.sems.allocated().values()))
            nc.all_engine_barrier()

    nc = bass.Bass(
        "TRN2", target_bir_lowering=False, debug=False, num_devices=NCORES
    )
    x_d = nc.dram_tensor("x", [C, P, FTOT], dt.bfloat16, kind="ExternalInput").ap()
    t_d = nc.dram_tensor("t", [P, FTOT], dt.bfloat16, kind="ExternalInput").ap()
    out_d = nc.dram_tensor("out", [C, 2], dt.float32, kind="ExternalOutput").ap()

    with nc.allow_low_precision("bf16 softmax-stat kernel"), \
            _TC(nc) as tc, ExitStack() as ctx:
        # DMA-written tiles get one slot per chunk: a DMACopy may carry at
        # most ONE sync-wait on TRN2, so slot reuse (which adds WAR/WAW
        # waits onto the DMA) must be avoided for them.
        xp = ctx.enter_context(tc.tile_pool(name="xp", bufs=3))
        ep = ctx.enter_context(tc.tile_pool(name="ep", bufs=2))
        tp = ctx.enter_context(tc.tile_pool(name="tp", bufs=NCHUNK))
        dp = ctx.enter_context(tc.tile_pool(name="dp", bufs=2))
        sp = ctx.enter_context(tc.tile_pool(name="sp", bufs=3))
        cp = ctx.enter_context(tc.tile_pool(name="cp", bufs=1))
        pp = ctx.enter_context(tc.tile_pool(name="pp", bufs=1, space="PSUM"))

        # per-class one-hot lhsT columns: block c is a [P, C] matrix whose
        # column c is all-ones -> matmul with rhs [P, F] lands the
        # pixel-partition sums of rhs on PSUM partition c.
        cols = cp.tile([P, C * C], dt.bfloat16)
        nc.gpsimd.memset(cols[:], 0.0)
        for c in range(C):
            nc.gpsimd.memset(cols[:, c * C + c : c * C + c + 1], 1.0)

        ps_acc = pp.tile([C, F], dt.float32)
        in_acc = pp.tile([C, F], dt.float32)

        for j in range(NCHUNK):
            # t first: it is tiny and the 19 mask ops depend only on it,
            # so DVE can chew masks while the big x transfer streams in.
            tt = tp.tile([P, F], dt.bfloat16, tag="t")
            nc.sync.dma_start(out=tt[:], in_=t_d[:, j * F : (j + 1) * F])
            ot = sp.tile([P, C * F], dt.bfloat16, tag="ot", bufs=1)
            for c in range(C):
                nc.vector.tensor_scalar(
                    ot[:, c * F : (c + 1) * F], tt[:], float(c), None, Alu.is_equal
                )
            xt = xp.tile([P, C * F], dt.bfloat16, tag="x")
            xv = xt[:].rearrange("p (c f) -> p c f", c=C)
            et = ep.tile([P, C * F], dt.bfloat16, tag="e")
            ev = et[:].rearrange("p (c f) -> p c f", c=C)
            CSPLIT = 10
            for c0, c1 in ((0, CSPLIT), (CSPLIT, C)):
                nc.sync.dma_start(
                    out=xv[:, c0:c1, :],
                    in_=x_d[c0:c1, :, j * F : (j + 1) * F].rearrange(
                        "c p f -> p c f"
                    ),
                )
                nc.scalar.activation(
                    et[:, c0 * F : c1 * F], xt[:, c0 * F : c1 * F], Act.Exp
                )

            # denominator: tree-sum the 19 class blocks (bf16 2x mode),
            # last add in fp32 for reciprocal_approx_fast
            s9 = sp.tile([P, 9 * F], dt.bfloat16, tag="s9", bufs=1)
            s9v = s9[:].rearrange("p (c f) -> p c f", c=9)
            nc.vector.tensor_tensor(
                s9v[:, :, :], ev[:, 0:18:2, :], ev[:, 1:19:2, :], Alu.add
            )
            s4 = sp.tile([P, 4 * F], dt.bfloat16, tag="s4", bufs=1)
            s4v = s4[:].rearrange("p (c f) -> p c f", c=4)
            nc.vector.tensor_tensor(
                s4v[:, :, :], s9v[:, 0:8:2, :], s9v[:, 1:9:2, :], Alu.add
            )
            s2 = sp.tile([P, 2 * F], dt.bfloat16, tag="s2", bufs=1)
            s2v = s2[:].rearrange("p (c f) -> p c f", c=2)
            nc.vector.tensor_tensor(
                s2v[:, :, :], s4v[:, 0:4:2, :], s4v[:, 1:4:2, :], Alu.add
            )
            s1 = sp.tile([P, F], dt.bfloat16, tag="s1", bufs=1)
            nc.vector.tensor_tensor(s1[:], s2v[:, 0, :], s2v[:, 1, :], Alu.add)
            d0 = sp.tile([P, F], dt.bfloat16, tag="d0", bufs=1)
            nc.vector.tensor_tensor(d0[:], s1[:], s9v[:, 8, :], Alu.add)
            dd = sp.tile([P, F], dt.bfloat16, tag="dd", bufs=1)
            nc.vector.tensor_tensor(dd[:], d0[:], ev[:, 18, :], Alu.add)
            rt = dp.tile([P, F], dt.bfloat16, tag="r")
            nc.vector.reciprocal(rt[:], dd[:])

            pa = sp.tile([P, C * F], dt.bfloat16, tag="pa", bufs=1)
            pav = pa[:].rearrange("p (c f) -> p c f", c=C)
            rb = rt[:].rearrange("p (o f) -> p o f", o=1).broadcast_to((P, C, F))
            nc.vector.tensor_tensor(pav[:, :, :], ev[:, :, :], rb, Alu.mult)
            oa = sp.tile([P, C * F], dt.bfloat16, tag="oa", bufs=1)
            nc.vector.tensor_tensor(oa[:], ot[:], pa[:], Alu.mult)
            for c in range(C):
                first = j == 0 and c == 0
                last = j == NCHUNK - 1 and c == C - 1
                lhs = cols[:, c * C : (c + 1) * C]
                nc.tensor.matmul(
                    ps_acc[:],
                    lhsT=lhs,
                    rhs=pa[:, c * F : (c + 1) * F],
                    start=first,
                    stop=last,
                )
                nc.tensor.matmul(
                    in_acc[:],
                    lhsT=lhs,
                    rhs=oa[:, c * F : (c + 1) * F],
                    start=first,
                    stop=last,
                )

        # free-dim reduce of the three PSUM accumulators -> [C, 3] -> DRAM
        ob = cp.tile([C, 2], dt.float32)
        for k, acc in enumerate((ps_acc, in_acc)):
            nc.vector.tensor_reduce(
                out=ob[:, k : k + 1],
                in_=acc[:],
                axis=mybir.AxisListType.X,
                op=Alu.add,
            )
        nc.sync.dma_start(out=out_d[:], in_=ob[:])

    _br.move_matmul_waits_to_ldweights(nc.m)
    _br.generate_event_semaphores(nc)
    return nc


def _get_program():
    global _PROG
    if _PROG is None:
        _PROG = _build_program()
    return _PROG


def _shard_inputs(predict, target):
    x = np.ascontiguousarray(predict, dtype=np.float32).reshape(N, C, P, FTOT)
    x = x.astype(ml_dtypes.bfloat16)
    t = (
        np.ascontiguousarray(target)
        .reshape(N, P, FTOT)
        .astype(np.float32)
        .astype(ml_dtypes.bfloat16)
    )
    return [{"x": x[i], "t": t[i]} for i in range(N)]


def kernel(predict, target):
    from concourse.bass_utils import run_bass_kernel_spmd

    nc = _get_program()
    in_maps = _shard_inputs(predict, target)
    res = run_bass_kernel_spmd(nc, in_maps, list(range(NCORES)))
    stats = np.stack(
        [np.asarray(res.results[i]["out"], dtype=np.float32).reshape(C, 2) for i in range(NCORES)]
    )
    psum = stats[:, :, 0]
    inter = stats[:, :, 1]
    tgt = np.ascontiguousarray(target).reshape(N, PIX)
    tsum = np.stack(
        [np.bincount(tgt[i].astype(np.int64), minlength=C)[:C] for i in range(N)]
    ).astype(np.float32)
    top = 2.0 * inter + 1.0
    bot = psum + tsum + 1.0
    per_class = np.mean(1.0 - top / bot, axis=0, dtype=np.float32)
    return np.float32(per_class.sum() / C)


# revision 33
# speedup vs baseline: 1.1756x; 1.0208x over previous
"""DiceLoss kernel for Trainium2 (8 NeuronCores, pure data parallel).

Problem: softmax over C=19 classes of predict [8, 19, 512, 512], one-hot of
target [8, 512, 512], then per-sample per-class sums
    psum[n,c]  = sum_pix softmax(x)[n,c,pix]
    inter[n,c] = sum_{pix: t=c} softmax(x)[n,c,pix]
    tsum[n,c]  = #{pix: t=c}
and dice = mean_c mean_n (1 - (2*inter+1)/(psum+tsum+1)).

Sharding: one sample per core (batch N=8 across 8 cores). Each core returns
[3*C] partial sums; the tiny final formula runs on host.

Device layout per core: x as [C, 128, 2048] bf16 (pixel-partition,
class-blocked free dim), processed in column chunks of F=512:
  - ScalarE: Exp activation (two class-group halves per chunk, pipelined
    behind the split DMA)
  - DVE: per-chunk one-hot masks (tensor_scalar is_equal, 4x mode, dep only
    on the tiny t tile so they fill the DMA/exp head), pairwise-tree class
    sum -> denominator (bf16 2x mode), reciprocal, then per class two bf16
    2x-mode products: P = E*R and OP = O*P
  - TensorE: per class a [128,19] one-hot-column lhsT matmul against
    rhs P / OP accumulates the pixel-partition sums for psum / inter into
    two [19, F] PSUM banks (start only on the very first matmul); a final
    free-dim reduce emits [C, 2] per core.
tsum is the exact integer histogram of the target input, computed on host
during sharding. Inputs are cast to bf16 on host (halves DMA bytes; fp32
PSUM accumulation keeps the loss error ~1e-6).

Hardware quirks worked around here: this walrus build allows at most ONE
sync-wait per instruction (two on InstEventSemaphore) -> tail-drain waits
are emitted as single-wait drains and the body is legalized with
bass_rust.generate_event_semaphores; InstISA-encoded DVE ops
(tensor_tensor_reduce, reciprocal_approx_*) fail codegen ("ISA wrong
length") and are avoided; gpsimd tensor ops measure ~10x slower than the
cost model and SWDGE DMAs add a ~30us Pool dge-drain to the tail, so all
DMAs go through SP HWDGE and gpsimd only does constant memsets.

Measured on trn2 via axon: HW exec ~123us per core (8 cores SPMD),
relative error vs fp32 reference ~6e-7. DVE-bound at ~88% occupancy.
"""

import numpy as np
import ml_dtypes

N, C, H, W = 8, 19, 512, 512
PIX = H * W  # 262144
P = 128
FTOT = PIX // P  # 2048
F = 512
NCHUNK = FTOT // F
NCORES = 8

_PROG = None


def _build_program():
    from contextlib import ExitStack

    import concourse.bass as bass
    import concourse.tile as tile
    from concourse import mybir

    dt = mybir.dt
    Alu = mybir.AluOpType
    Act = mybir.ActivationFunctionType

    import bass_rust as _br

    class _TC(tile.TileContext):
        # Stock Tile puts one sem-wait per active proc on the tail drain,
        # which this walrus rejects (>1 wait per instruction). Emit the
        # global-clock waits as single-wait drains instead; body
        # instructions are legalized by bass_rust.generate_event_semaphores
        # after the context exits.
        def _drain_and_barrier(self, tick_clock, wait_clock):
            from concourse.vector_clock import ScopedClock

            nc = self.nc
            drain_inst = nc.sync.drain()
            wait_clock.add_sem_waits(
                drain_inst.ins, ScopedClock({None: tick_clock.global_clock})
            )
            si = drain_inst.ins.sync_info
            moved = []
            while len(si.on_wait) > 1:
                moved.append(si.on_wait.pop())
            for w in moved:
                d2 = nc.sync.drain()
                d2.ins.sync_info = _br.SyncInfo(on_wait=[w], on_update=[])

            nc.all_engine_barrier()
            assert self.sems is not None
            popped = nc._tile_sem_poison_stack.pop()
            assert popped is self._sem_poison
            nc.clear_and_free_semaphores(list(self.sems.allocated().values()))
            nc.all_engine_barrier()

    nc = bass.Bass(
        "TRN2", target_bir_lowering=False, debug=False, num_devices=NCORES
    )
    x_d = nc.dram_tensor("x", [C, P, FTOT], dt.bfloat16, kind="ExternalInput").ap()
    t_d = nc.dram_tensor("t", [P, FTOT], dt.bfloat16, kind="ExternalInput").ap()
    out_d = nc.dram_tensor("out", [C, 2], dt.float32, kind="ExternalOutput").ap()

    with nc.allow_low_precision("bf16 softmax-stat kernel"), \
            _TC(nc) as tc, ExitStack() as ctx:
        # DMA-written tiles get one slot per chunk: a DMACopy may carry at
        # most ONE sync-wait on TRN2, so slot reuse (which adds WAR/WAW
        # waits onto the DMA) must be avoided for them.
        xp = ctx.enter_context(tc.tile_pool(name="xp", bufs=3))
        ep = ctx.enter_context(tc.tile_pool(name="ep", bufs=2))
        tp = ctx.enter_context(tc.tile_pool(name="tp", bufs=NCHUNK))
        dp = ctx.enter_context(tc.tile_pool(name="dp", bufs=2))
        sp = ctx.enter_context(tc.tile_pool(name="sp", bufs=3))
        cp = ctx.enter_context(tc.tile_pool(name="cp", bufs=1))
        pp = ctx.enter_context(tc.tile_pool(name="pp", bufs=1, space="PSUM"))

        # per-class one-hot lhsT columns: block c is a [P, C] matrix whose
        # column c is all-ones -> matmul with rhs [P, F] lands the
        # pixel-partition sums of rhs on PSUM partition c.
        cols = cp.tile([P, C * C], dt.bfloat16)
        nc.gpsimd.memset(cols[:], 0.0)
        for c in range(C):
            nc.gpsimd.memset(cols[:, c * C + c : c * C + c + 1], 1.0)

        ps_acc = pp.tile([C, F], dt.float32)
        in_acc = pp.tile([C, F], dt.float32)

        for j in range(NCHUNK):
            # t first: it is tiny and the 19 mask ops depend only on it,
            # so DVE can chew masks while the big x transfer streams in.
            tt = tp.tile([P, F], dt.bfloat16, tag="t")
            nc.sync.dma_start(out=tt[:], in_=t_d[:, j * F : (j + 1) * F])
            ot = sp.tile([P, C * F], dt.bfloat16, tag="ot", bufs=2)
            for c in range(C):
                nc.vector.tensor_scalar(
                    ot[:, c * F : (c + 1) * F], tt[:], float(c), None, Alu.is_equal
                )
            xt = xp.tile([P, C * F], dt.bfloat16, tag="x")
            xv = xt[:].rearrange("p (c f) -> p c f", c=C)
            et = ep.tile([P, C * F], dt.bfloat16, tag="e")
            ev = et[:].rearrange("p (c f) -> p c f", c=C)
            CSPLIT = 10
            for c0, c1 in ((0, CSPLIT), (CSPLIT, C)):
                nc.sync.dma_start(
                    out=xv[:, c0:c1, :],
                    in_=x_d[c0:c1, :, j * F : (j + 1) * F].rearrange(
                        "c p f -> p c f"
                    ),
                )
                nc.scalar.activation(
                    et[:, c0 * F : c1 * F], xt[:, c0 * F : c1 * F], Act.Exp
                )

            # denominator: tree-sum the 19 class blocks (bf16 2x mode),
            # last add in fp32 for reciprocal_approx_fast
            s9 = sp.tile([P, 9 * F], dt.bfloat16, tag="s9", bufs=1)
            s9v = s9[:].rearrange("p (c f) -> p c f", c=9)
            nc.vector.tensor_tensor(
                s9v[:, :, :], ev[:, 0:18:2, :], ev[:, 1:19:2, :], Alu.add
            )
            s4 = sp.tile([P, 4 * F], dt.bfloat16, tag="s4", bufs=1)
            s4v = s4[:].rearrange("p (c f) -> p c f", c=4)
            nc.vector.tensor_tensor(
                s4v[:, :, :], s9v[:, 0:8:2, :], s9v[:, 1:9:2, :], Alu.add
            )
            s2 = sp.tile([P, 2 * F], dt.bfloat16, tag="s2", bufs=1)
            s2v = s2[:].rearrange("p (c f) -> p c f", c=2)
            nc.vector.tensor_tensor(
                s2v[:, :, :], s4v[:, 0:4:2, :], s4v[:, 1:4:2, :], Alu.add
            )
            s1 = sp.tile([P, F], dt.bfloat16, tag="s1", bufs=1)
            nc.vector.tensor_tensor(s1[:], s2v[:, 0, :], s2v[:, 1, :], Alu.add)
            d0 = sp.tile([P, F], dt.bfloat16, tag="d0", bufs=1)
            nc.vector.tensor_tensor(d0[:], s1[:], s9v[:, 8, :], Alu.add)
            dd = sp.tile([P, F], dt.bfloat16, tag="dd", bufs=1)
            nc.vector.tensor_tensor(dd[:], d0[:], ev[:, 18, :], Alu.add)
            rt = dp.tile([P, F], dt.bfloat16, tag="r")
            nc.vector.reciprocal(rt[:], dd[:])

            # in-place wide products: E is dead after P=E*R, O after OP=O*P,
            # so overwrite et with P and ot with OP — no extra tiles, and the
            # 2-deep et/ot rings absorb the PE read lag across chunks.
            rb = rt[:].rearrange("p (o f) -> p o f", o=1).broadcast_to((P, C, F))
            nc.vector.tensor_tensor(ev[:, :, :], ev[:, :, :], rb, Alu.mult)
            nc.vector.tensor_tensor(ot[:], ot[:], et[:], Alu.mult)
            for c in range(C):
                first = j == 0 and c == 0
                last = j == NCHUNK - 1 and c == C - 1
                lhs = cols[:, c * C : (c + 1) * C]
                nc.tensor.matmul(
                    ps_acc[:],
                    lhsT=lhs,
                    rhs=et[:, c * F : (c + 1) * F],
                    start=first,
                    stop=last,
                )
                nc.tensor.matmul(
                    in_acc[:],
                    lhsT=lhs,
                    rhs=ot[:, c * F : (c + 1) * F],
                    start=first,
                    stop=last,
                )

        # free-dim reduce of the three PSUM accumulators -> [C, 3] -> DRAM
        ob = cp.tile([C, 2], dt.float32)
        for k, acc in enumerate((ps_acc, in_acc)):
            nc.vector.tensor_reduce(
                out=ob[:, k : k + 1],
                in_=acc[:],
                axis=mybir.AxisListType.X,
                op=Alu.add,
            )
        nc.sync.dma_start(out=out_d[:], in_=ob[:])

    _br.move_matmul_waits_to_ldweights(nc.m)
    _br.generate_event_semaphores(nc)
    return nc


def _get_program():
    global _PROG
    if _PROG is None:
        _PROG = _build_program()
    return _PROG


def _shard_inputs(predict, target):
    x = np.ascontiguousarray(predict, dtype=np.float32).reshape(N, C, P, FTOT)
    x = x.astype(ml_dtypes.bfloat16)
    t = (
        np.ascontiguousarray(target)
        .reshape(N, P, FTOT)
        .astype(np.float32)
        .astype(ml_dtypes.bfloat16)
    )
    return [{"x": x[i], "t": t[i]} for i in range(N)]


def kernel(predict, target):
    from concourse.bass_utils import run_bass_kernel_spmd

    nc = _get_program()
    in_maps = _shard_inputs(predict, target)
    res = run_bass_kernel_spmd(nc, in_maps, list(range(NCORES)))
    stats = np.stack(
        [np.asarray(res.results[i]["out"], dtype=np.float32).reshape(C, 2) for i in range(NCORES)]
    )
    psum = stats[:, :, 0]
    inter = stats[:, :, 1]
    tgt = np.ascontiguousarray(target).reshape(N, PIX)
    tsum = np.stack(
        [np.bincount(tgt[i].astype(np.int64), minlength=C)[:C] for i in range(N)]
    ).astype(np.float32)
    top = 2.0 * inter + 1.0
    bot = psum + tsum + 1.0
    per_class = np.mean(1.0 - top / bot, axis=0, dtype=np.float32)
    return np.float32(per_class.sum() / C)


# revision 35
# speedup vs baseline: 1.1814x; 1.0050x over previous
"""DiceLoss kernel for Trainium2 (8 NeuronCores, pure data parallel).

Problem: softmax over C=19 classes of predict [8, 19, 512, 512], one-hot of
target [8, 512, 512], then per-sample per-class sums
    psum[n,c]  = sum_pix softmax(x)[n,c,pix]
    inter[n,c] = sum_{pix: t=c} softmax(x)[n,c,pix]
    tsum[n,c]  = #{pix: t=c}
and dice = mean_c mean_n (1 - (2*inter+1)/(psum+tsum+1)).

Sharding: one sample per core (batch N=8 across 8 cores). Each core returns
[3*C] partial sums; the tiny final formula runs on host.

Device layout per core: x as [C, 128, 2048] bf16 (pixel-partition,
class-blocked free dim), processed in column chunks of F=512:
  - ScalarE: Exp activation (two class-group halves per chunk, pipelined
    behind the split DMA)
  - DVE: per-chunk one-hot masks (tensor_scalar is_equal, 4x mode, dep only
    on the tiny t tile so they fill the DMA/exp head), pairwise-tree class
    sum -> denominator (bf16 2x mode), reciprocal, then TWO chunk-wide bf16
    2x-mode in-place products: et *= R (broadcast) and ot *= et — one
    instruction each over all 19 classes (E and O are dead afterwards, so
    overwriting them costs no SBUF and the 2-deep rings absorb PE read lag)
  - TensorE: per class a [128,19] one-hot-column lhsT matmul against
    rhs P / OP accumulates the pixel-partition sums for psum / inter into
    two [19, F] PSUM banks (start only on the very first matmul); a final
    free-dim reduce emits [C, 2] per core.
tsum is the exact integer histogram of the target input, computed on host
during sharding. Inputs are cast to bf16 on host (halves DMA bytes; fp32
PSUM accumulation keeps the loss error ~1e-6).

Hardware quirks worked around here: this walrus build allows at most ONE
sync-wait per instruction (two on InstEventSemaphore) -> tail-drain waits
are emitted as single-wait drains and the body is legalized with
bass_rust.generate_event_semaphores; InstISA-encoded DVE ops
(tensor_tensor_reduce, reciprocal_approx_*) fail codegen ("ISA wrong
length") and are avoided; gpsimd tensor ops measure ~10x slower than the
cost model and SWDGE DMAs add a ~30us Pool dge-drain to the tail, so all
DMAs go through SP HWDGE and gpsimd only does constant memsets.

Measured on trn2 via axon: HW exec ~119us per core (8 cores SPMD),
relative error vs fp32 reference ~6e-7. DVE-bound at ~83% occupancy.
"""

import numpy as np
import ml_dtypes

N, C, H, W = 8, 19, 512, 512
PIX = H * W  # 262144
P = 128
FTOT = PIX // P  # 2048
F = 512
NCHUNK = FTOT // F
NCORES = 8

_PROG = None


def _build_program():
    from contextlib import ExitStack

    import concourse.bass as bass
    import concourse.tile as tile
    from concourse import mybir

    dt = mybir.dt
    Alu = mybir.AluOpType
    Act = mybir.ActivationFunctionType

    import bass_rust as _br

    class _TC(tile.TileContext):
        # Stock Tile puts one sem-wait per active proc on the tail drain,
        # which this walrus rejects (>1 wait per instruction). Emit the
        # global-clock waits as single-wait drains instead; body
        # instructions are legalized by bass_rust.generate_event_semaphores
        # after the context exits.
        def _drain_and_barrier(self, tick_clock, wait_clock):
            from concourse.vector_clock import ScopedClock

            nc = self.nc
            drain_inst = nc.sync.drain()
            wait_clock.add_sem_waits(
                drain_inst.ins, ScopedClock({None: tick_clock.global_clock})
            )
            si = drain_inst.ins.sync_info
            moved = []
            while len(si.on_wait) > 1:
                moved.append(si.on_wait.pop())
            for w in moved:
                d2 = nc.sync.drain()
                d2.ins.sync_info = _br.SyncInfo(on_wait=[w], on_update=[])

            nc.all_engine_barrier()
            assert self.sems is not None
            popped = nc._tile_sem_poison_stack.pop()
            assert popped is self._sem_poison
            nc.clear_and_free_semaphores(list(self.sems.allocated().values()))
            nc.all_engine_barrier()

    nc = bass.Bass(
        "TRN2", target_bir_lowering=False, debug=False, num_devices=NCORES
    )
    x_d = nc.dram_tensor("x", [C, P, FTOT], dt.bfloat16, kind="ExternalInput").ap()
    t_d = nc.dram_tensor("t", [P, FTOT], dt.bfloat16, kind="ExternalInput").ap()
    out_d = nc.dram_tensor("out", [C, 2], dt.float32, kind="ExternalOutput").ap()

    with nc.allow_low_precision("bf16 softmax-stat kernel"), \
            _TC(nc) as tc, ExitStack() as ctx:
        # DMA-written tiles get one slot per chunk: a DMACopy may carry at
        # most ONE sync-wait on TRN2, so slot reuse (which adds WAR/WAW
        # waits onto the DMA) must be avoided for them.
        xp = ctx.enter_context(tc.tile_pool(name="xp", bufs=3))
        ep = ctx.enter_context(tc.tile_pool(name="ep", bufs=2))
        tp = ctx.enter_context(tc.tile_pool(name="tp", bufs=NCHUNK))
        dp = ctx.enter_context(tc.tile_pool(name="dp", bufs=2))
        sp = ctx.enter_context(tc.tile_pool(name="sp", bufs=3))
        cp = ctx.enter_context(tc.tile_pool(name="cp", bufs=1))
        pp = ctx.enter_context(tc.tile_pool(name="pp", bufs=1, space="PSUM"))

        # per-class one-hot lhsT columns: block c is a [P, C] matrix whose
        # column c is all-ones -> matmul with rhs [P, F] lands the
        # pixel-partition sums of rhs on PSUM partition c.
        cols = cp.tile([P, C * C], dt.bfloat16)
        nc.gpsimd.memset(cols[:], 0.0)
        for c in range(C):
            nc.gpsimd.memset(cols[:, c * C + c : c * C + c + 1], 1.0)

        ps_acc = pp.tile([C, F], dt.float32)
        in_acc = pp.tile([C, F], dt.float32)

        for j in range(NCHUNK):
            # t first: it is tiny and the 19 mask ops depend only on it,
            # so DVE can chew masks while the big x transfer streams in.
            tt = tp.tile([P, F], dt.bfloat16, tag="t")
            nc.sync.dma_start(out=tt[:], in_=t_d[:, j * F : (j + 1) * F])
            ot = sp.tile([P, C * F], dt.bfloat16, tag="ot", bufs=2)
            for c in range(C):
                nc.vector.tensor_scalar(
                    ot[:, c * F : (c + 1) * F], tt[:], float(c), None, Alu.is_equal
                )
            xt = xp.tile([P, C * F], dt.bfloat16, tag="x")
            xv = xt[:].rearrange("p (c f) -> p c f", c=C)
            et = ep.tile([P, C * F], dt.bfloat16, tag="e")
            ev = et[:].rearrange("p (c f) -> p c f", c=C)
            CSPLIT = 10
            for c0, c1 in ((0, CSPLIT), (CSPLIT, C)):
                nc.sync.dma_start(
                    out=xv[:, c0:c1, :],
                    in_=x_d[c0:c1, :, j * F : (j + 1) * F].rearrange(
                        "c p f -> p c f"
                    ),
                )
                nc.scalar.activation(
                    et[:, c0 * F : c1 * F], xt[:, c0 * F : c1 * F], Act.Exp
                )

            # denominator: tree-sum split by exp half so level 1 of the
            # first 10 classes runs while exp of classes 10-18 is still going
            sa = sp.tile([P, 5 * F], dt.bfloat16, tag="sa", bufs=1)
            sav = sa[:].rearrange("p (c f) -> p c f", c=5)
            nc.vector.tensor_tensor(
                sav[:, :, :], ev[:, 0:10:2, :], ev[:, 1:10:2, :], Alu.add
            )
            sb = sp.tile([P, 4 * F], dt.bfloat16, tag="sb", bufs=1)
            sbv = sb[:].rearrange("p (c f) -> p c f", c=4)
            nc.vector.tensor_tensor(
                sbv[:, :, :], ev[:, 10:18:2, :], ev[:, 11:19:2, :], Alu.add
            )
            sc = sp.tile([P, 2 * F], dt.bfloat16, tag="sc", bufs=1)
            scv = sc[:].rearrange("p (c f) -> p c f", c=2)
            nc.vector.tensor_tensor(
                scv[:, :, :], sav[:, 0:4:2, :], sav[:, 1:5:2, :], Alu.add
            )
            sd = sp.tile([P, 2 * F], dt.bfloat16, tag="sd", bufs=1)
            sdv = sd[:].rearrange("p (c f) -> p c f", c=2)
            nc.vector.tensor_tensor(
                sdv[:, :, :], sbv[:, 0:4:2, :], sbv[:, 1:4:2, :], Alu.add
            )
            se = sp.tile([P, F], dt.bfloat16, tag="se", bufs=1)
            nc.vector.tensor_tensor(se[:], scv[:, 0, :], scv[:, 1, :], Alu.add)
            sf = sp.tile([P, F], dt.bfloat16, tag="sf", bufs=1)
            nc.vector.tensor_tensor(sf[:], sdv[:, 0, :], sdv[:, 1, :], Alu.add)
            d0 = sp.tile([P, F], dt.bfloat16, tag="d0", bufs=1)
            nc.vector.tensor_tensor(d0[:], se[:], sf[:], Alu.add)
            d1 = sp.tile([P, F], dt.bfloat16, tag="d1", bufs=1)
            nc.vector.tensor_tensor(d1[:], d0[:], sav[:, 4, :], Alu.add)
            dd = sp.tile([P, F], dt.bfloat16, tag="dd", bufs=1)
            nc.vector.tensor_tensor(dd[:], d1[:], ev[:, 18, :], Alu.add)
            rt = dp.tile([P, F], dt.bfloat16, tag="r")
            nc.vector.reciprocal(rt[:], dd[:])

            # in-place wide products: E is dead after P=E*R, O after OP=O*P,
            # so overwrite et with P and ot with OP — no extra tiles, and the
            # 2-deep et/ot rings absorb the PE read lag across chunks.
            rb = rt[:].rearrange("p (o f) -> p o f", o=1).broadcast_to((P, C, F))
            nc.vector.tensor_tensor(ev[:, :, :], ev[:, :, :], rb, Alu.mult)
            nc.vector.tensor_tensor(ot[:], ot[:], et[:], Alu.mult)
            for c in range(C):
                first = j == 0 and c == 0
                last = j == NCHUNK - 1 and c == C - 1
                lhs = cols[:, c * C : (c + 1) * C]
                nc.tensor.matmul(
                    ps_acc[:],
                    lhsT=lhs,
                    rhs=et[:, c * F : (c + 1) * F],
                    start=first,
                    stop=last,
                )
                nc.tensor.matmul(
                    in_acc[:],
                    lhsT=lhs,
                    rhs=ot[:, c * F : (c + 1) * F],
                    start=first,
                    stop=last,
                )

        # free-dim reduce of the three PSUM accumulators -> [C, 3] -> DRAM
        ob = cp.tile([C, 2], dt.float32)
        for k, acc in enumerate((ps_acc, in_acc)):
            nc.vector.tensor_reduce(
                out=ob[:, k : k + 1],
                in_=acc[:],
                axis=mybir.AxisListType.X,
                op=Alu.add,
            )
        nc.sync.dma_start(out=out_d[:], in_=ob[:])

    _br.move_matmul_waits_to_ldweights(nc.m)
    _br.generate_event_semaphores(nc)
    return nc


def _get_program():
    global _PROG
    if _PROG is None:
        _PROG = _build_program()
    return _PROG


def _shard_inputs(predict, target):
    x = np.ascontiguousarray(predict, dtype=np.float32).reshape(N, C, P, FTOT)
    x = x.astype(ml_dtypes.bfloat16)
    t = (
        np.ascontiguousarray(target)
        .reshape(N, P, FTOT)
        .astype(np.float32)
        .astype(ml_dtypes.bfloat16)
    )
    return [{"x": x[i], "t": t[i]} for i in range(N)]


def kernel(predict, target):
    from concourse.bass_utils import run_bass_kernel_spmd

    nc = _get_program()
    in_maps = _shard_inputs(predict, target)
    res = run_bass_kernel_spmd(nc, in_maps, list(range(NCORES)))
    stats = np.stack(
        [np.asarray(res.results[i]["out"], dtype=np.float32).reshape(C, 2) for i in range(NCORES)]
    )
    psum = stats[:, :, 0]
    inter = stats[:, :, 1]
    tgt = np.ascontiguousarray(target).reshape(N, PIX)
    tsum = np.stack(
        [np.bincount(tgt[i].astype(np.int64), minlength=C)[:C] for i in range(N)]
    ).astype(np.float32)
    top = 2.0 * inter + 1.0
    bot = psum + tsum + 1.0
    per_class = np.mean(1.0 - top / bot, axis=0, dtype=np.float32)
    return np.float32(per_class.sum() / C)


# revision 37
# speedup vs baseline: 1.1921x; 1.0090x over previous
"""DiceLoss kernel for Trainium2 (8 NeuronCores, pure data parallel).

Problem: softmax over C=19 classes of predict [8, 19, 512, 512], one-hot of
target [8, 512, 512], then per-sample per-class sums
    psum[n,c]  = sum_pix softmax(x)[n,c,pix]
    inter[n,c] = sum_{pix: t=c} softmax(x)[n,c,pix]
    tsum[n,c]  = #{pix: t=c}
and dice = mean_c mean_n (1 - (2*inter+1)/(psum+tsum+1)).

Sharding: one sample per core (batch N=8 across 8 cores). Each core returns
[3*C] partial sums; the tiny final formula runs on host.

Device layout per core: x as [C, 128, 2048] bf16 (pixel-partition,
class-blocked free dim), processed in column chunks of F=512:
  - ScalarE: Exp activation (two class-group halves per chunk, pipelined
    behind the split DMA)
  - DVE: per-chunk one-hot masks (tensor_scalar is_equal, 4x mode, dep only
    on the tiny t tile so they fill the DMA/exp head), pairwise-tree class
    sum -> denominator (bf16 2x mode, level 1 split by exp half so it starts
    while the second half is still exponentiating), reciprocal, then TWO
    chunk-wide bf16
    2x-mode in-place products: et *= R (broadcast) and ot *= et — one
    instruction each over all 19 classes (E and O are dead afterwards, so
    overwriting them costs no SBUF and the 2-deep rings absorb PE read lag)
  - TensorE: per class a [128,19] one-hot-column lhsT matmul against
    rhs P / OP accumulates the pixel-partition sums for psum / inter into
    two [19, F] PSUM banks (start only on the very first matmul); a final
    free-dim reduce emits [C, 2] per core.
tsum is the exact integer histogram of the target input, computed on host
during sharding. Inputs are cast to bf16 on host (halves DMA bytes; fp32
PSUM accumulation keeps the loss error ~1e-6).

Hardware quirks worked around here: this walrus build allows at most ONE
sync-wait per instruction (two on InstEventSemaphore) -> tail-drain waits
are emitted as single-wait drains and the body is legalized with
bass_rust.generate_event_semaphores; InstISA-encoded DVE ops
(tensor_tensor_reduce, reciprocal_approx_*) fail codegen ("ISA wrong
length") and are avoided; gpsimd tensor ops measure ~10x slower than the
cost model and SWDGE DMAs add a ~30us Pool dge-drain to the tail, so all
DMAs go through SP HWDGE and gpsimd only does constant memsets.

Measured on trn2 via axon: HW exec ~118.6us per core (8 cores SPMD),
relative error vs fp32 reference ~8e-7. DVE-bound at ~84% occupancy.
"""

import numpy as np
import ml_dtypes

N, C, H, W = 8, 19, 512, 512
PIX = H * W  # 262144
P = 128
FTOT = PIX // P  # 2048
F = 512
NCHUNK = FTOT // F
NCORES = 8

_PROG = None


def _build_program():
    from contextlib import ExitStack

    import concourse.bass as bass
    import concourse.tile as tile
    from concourse import mybir

    dt = mybir.dt
    Alu = mybir.AluOpType
    Act = mybir.ActivationFunctionType

    import bass_rust as _br

    class _TC(tile.TileContext):
        # Stock Tile puts one sem-wait per active proc on the tail drain,
        # which this walrus rejects (>1 wait per instruction). Emit the
        # global-clock waits as single-wait drains instead; body
        # instructions are legalized by bass_rust.generate_event_semaphores
        # after the context exits.
        def _drain_and_barrier(self, tick_clock, wait_clock):
            from concourse.vector_clock import ScopedClock

            nc = self.nc
            drain_inst = nc.sync.drain()
            wait_clock.add_sem_waits(
                drain_inst.ins, ScopedClock({None: tick_clock.global_clock})
            )
            si = drain_inst.ins.sync_info
            moved = []
            while len(si.on_wait) > 1:
                moved.append(si.on_wait.pop())
            for w in moved:
                d2 = nc.sync.drain()
                d2.ins.sync_info = _br.SyncInfo(on_wait=[w], on_update=[])

            nc.all_engine_barrier()
            assert self.sems is not None
            popped = nc._tile_sem_poison_stack.pop()
            assert popped is self._sem_poison
            nc.clear_and_free_semaphores(list(self.sems.allocated().values()))
            nc.all_engine_barrier()

    nc = bass.Bass(
        "TRN2", target_bir_lowering=False, debug=False, num_devices=NCORES
    )
    x_d = nc.dram_tensor("x", [C, P, FTOT], dt.bfloat16, kind="ExternalInput").ap()
    t_d = nc.dram_tensor("t", [P, FTOT], dt.bfloat16, kind="ExternalInput").ap()
    out_d = nc.dram_tensor("out", [C, 2], dt.float32, kind="ExternalOutput").ap()

    with nc.allow_low_precision("bf16 softmax-stat kernel"), \
            _TC(nc) as tc, ExitStack() as ctx:
        # DMA-written tiles get one slot per chunk: a DMACopy may carry at
        # most ONE sync-wait on TRN2, so slot reuse (which adds WAR/WAW
        # waits onto the DMA) must be avoided for them.
        xp = ctx.enter_context(tc.tile_pool(name="xp", bufs=3))
        ep = ctx.enter_context(tc.tile_pool(name="ep", bufs=2))
        tp = ctx.enter_context(tc.tile_pool(name="tp", bufs=NCHUNK))
        dp = ctx.enter_context(tc.tile_pool(name="dp", bufs=2))
        sp = ctx.enter_context(tc.tile_pool(name="sp", bufs=3))
        cp = ctx.enter_context(tc.tile_pool(name="cp", bufs=1))
        pp = ctx.enter_context(tc.tile_pool(name="pp", bufs=1, space="PSUM"))

        # per-class one-hot lhsT columns: block c is a [P, C] matrix whose
        # column c is all-ones -> matmul with rhs [P, F] lands the
        # pixel-partition sums of rhs on PSUM partition c.
        cols = cp.tile([P, C * C], dt.bfloat16)
        nc.gpsimd.memset(cols[:], 0.0)
        for c in range(C):
            nc.gpsimd.memset(cols[:, c * C + c : c * C + c + 1], 1.0)

        ps_acc = pp.tile([C, F], dt.float32)
        in_acc = pp.tile([C, F], dt.float32)

        # all four t slices up front on the ACT HWDGE queue (parallel to the
        # big x transfers on SP): every chunk's mask batch becomes available
        # within ~2us, giving DVE gap-filler work for the whole pipeline.
        tts = []
        for j in range(NCHUNK):
            tt = tp.tile([P, F], dt.bfloat16, tag="t", name=f"tt{j}")
            nc.scalar.dma_start(out=tt[:], in_=t_d[:, j * F : (j + 1) * F])
            tts.append(tt)

        for j in range(NCHUNK):
            tt = tts[j]
            ot = sp.tile([P, C * F], dt.bfloat16, tag="ot", bufs=2)
            for c in range(C):
                nc.vector.tensor_scalar(
                    ot[:, c * F : (c + 1) * F], tt[:], float(c), None, Alu.is_equal
                )
            xt = xp.tile([P, C * F], dt.bfloat16, tag="x")
            xv = xt[:].rearrange("p (c f) -> p c f", c=C)
            et = ep.tile([P, C * F], dt.bfloat16, tag="e")
            ev = et[:].rearrange("p (c f) -> p c f", c=C)
            CSPLIT = 10
            for c0, c1 in ((0, CSPLIT), (CSPLIT, C)):
                nc.sync.dma_start(
                    out=xv[:, c0:c1, :],
                    in_=x_d[c0:c1, :, j * F : (j + 1) * F].rearrange(
                        "c p f -> p c f"
                    ),
                )
                nc.scalar.activation(
                    et[:, c0 * F : c1 * F], xt[:, c0 * F : c1 * F], Act.Exp
                )

            # denominator: tree-sum split by exp half so level 1 of the
            # first 10 classes runs while exp of classes 10-18 is still going
            sa = sp.tile([P, 5 * F], dt.bfloat16, tag="sa", bufs=1)
            sav = sa[:].rearrange("p (c f) -> p c f", c=5)
            nc.vector.tensor_tensor(
                sav[:, :, :], ev[:, 0:10:2, :], ev[:, 1:10:2, :], Alu.add
            )
            sb = sp.tile([P, 4 * F], dt.bfloat16, tag="sb", bufs=1)
            sbv = sb[:].rearrange("p (c f) -> p c f", c=4)
            nc.vector.tensor_tensor(
                sbv[:, :, :], ev[:, 10:18:2, :], ev[:, 11:19:2, :], Alu.add
            )
            sc = sp.tile([P, 2 * F], dt.bfloat16, tag="sc", bufs=1)
            scv = sc[:].rearrange("p (c f) -> p c f", c=2)
            nc.vector.tensor_tensor(
                scv[:, :, :], sav[:, 0:4:2, :], sav[:, 1:5:2, :], Alu.add
            )
            sd = sp.tile([P, 2 * F], dt.bfloat16, tag="sd", bufs=1)
            sdv = sd[:].rearrange("p (c f) -> p c f", c=2)
            nc.vector.tensor_tensor(
                sdv[:, :, :], sbv[:, 0:4:2, :], sbv[:, 1:4:2, :], Alu.add
            )
            se = sp.tile([P, F], dt.bfloat16, tag="se", bufs=1)
            nc.vector.tensor_tensor(se[:], scv[:, 0, :], scv[:, 1, :], Alu.add)
            sf = sp.tile([P, F], dt.bfloat16, tag="sf", bufs=1)
            nc.vector.tensor_tensor(sf[:], sdv[:, 0, :], sdv[:, 1, :], Alu.add)
            d0 = sp.tile([P, F], dt.bfloat16, tag="d0", bufs=1)
            nc.vector.tensor_tensor(d0[:], se[:], sf[:], Alu.add)
            d1 = sp.tile([P, F], dt.bfloat16, tag="d1", bufs=1)
            nc.vector.tensor_tensor(d1[:], d0[:], sav[:, 4, :], Alu.add)
            dd = sp.tile([P, F], dt.bfloat16, tag="dd", bufs=1)
            nc.vector.tensor_tensor(dd[:], d1[:], ev[:, 18, :], Alu.add)
            rt = dp.tile([P, F], dt.bfloat16, tag="r")
            nc.vector.reciprocal(rt[:], dd[:])

            # in-place wide products: E is dead after P=E*R, O after OP=O*P,
            # so overwrite et with P and ot with OP — no extra tiles, and the
            # 2-deep et/ot rings absorb the PE read lag across chunks.
            rb = rt[:].rearrange("p (o f) -> p o f", o=1).broadcast_to((P, C, F))
            nc.vector.tensor_tensor(ev[:, :, :], ev[:, :, :], rb, Alu.mult)
            nc.vector.tensor_tensor(ot[:], ot[:], et[:], Alu.mult)
            for c in range(C):
                first = j == 0 and c == 0
                last = j == NCHUNK - 1 and c == C - 1
                lhs = cols[:, c * C : (c + 1) * C]
                nc.tensor.matmul(
                    ps_acc[:],
                    lhsT=lhs,
                    rhs=et[:, c * F : (c + 1) * F],
                    start=first,
                    stop=last,
                )
                nc.tensor.matmul(
                    in_acc[:],
                    lhsT=lhs,
                    rhs=ot[:, c * F : (c + 1) * F],
                    start=first,
                    stop=last,
                )

        # free-dim reduce of the three PSUM accumulators -> [C, 3] -> DRAM
        ob = cp.tile([C, 2], dt.float32)
        for k, acc in enumerate((ps_acc, in_acc)):
            nc.vector.tensor_reduce(
                out=ob[:, k : k + 1],
                in_=acc[:],
                axis=mybir.AxisListType.X,
                op=Alu.add,
            )
        nc.sync.dma_start(out=out_d[:], in_=ob[:])

    _br.move_matmul_waits_to_ldweights(nc.m)
    _br.generate_event_semaphores(nc)
    return nc


def _get_program():
    global _PROG
    if _PROG is None:
        _PROG = _build_program()
    return _PROG


def _shard_inputs(predict, target):
    x = np.ascontiguousarray(predict, dtype=np.float32).reshape(N, C, P, FTOT)
    x = x.astype(ml_dtypes.bfloat16)
    t = (
        np.ascontiguousarray(target)
        .reshape(N, P, FTOT)
        .astype(np.float32)
        .astype(ml_dtypes.bfloat16)
    )
    return [{"x": x[i], "t": t[i]} for i in range(N)]


def kernel(predict, target):
    from concourse.bass_utils import run_bass_kernel_spmd

    nc = _get_program()
    in_maps = _shard_inputs(predict, target)
    res = run_bass_kernel_spmd(nc, in_maps, list(range(NCORES)))
    stats = np.stack(
        [np.asarray(res.results[i]["out"], dtype=np.float32).reshape(C, 2) for i in range(NCORES)]
    )
    psum = stats[:, :, 0]
    inter = stats[:, :, 1]
    tgt = np.ascontiguousarray(target).reshape(N, PIX)
    tsum = np.stack(
        [np.bincount(tgt[i].astype(np.int64), minlength=C)[:C] for i in range(N)]
    ).astype(np.float32)
    top = 2.0 * inter + 1.0
    bot = psum + tsum + 1.0
    per_class = np.mean(1.0 - top / bot, axis=0, dtype=np.float32)
    return np.float32(per_class.sum() / C)
